# revision 26
# baseline (speedup 1.0000x reference)
"""Bahdanau additive attention kernel for Trainium2 (8 NeuronCores, SPMD).

Problem (hardcoded): B=32, Tq=4, S=2048, H=1024, 2H=2048, fp32 inputs.
  q  = query[:, -1, :]                      [B, H]
  k  = transpose(keys, (1, 0, 2))           [B, S, 2H]
  wq = q @ Wa_w.T + Wa_b                    [B, H]
  uk = k @ Ua_w.T + Ua_b                    [B, S, H]
  sc = tanh(wq[:, None, :] + uk) @ Va_w.T   [B, S]   (+ Va_b, which softmax cancels)
  w  = softmax(sc, axis=-1)                 [B, S]
  ctx = w @ k                               [B, 2H]
  returns (ctx [B,1,2H], w [B,1,S])

Sharding: data-parallel over batch. 8 cores x 4 batches each; weights
replicated; no cross-core communication.

Host prep: weights are pre-transposed and cast to bf16 on the host
(uaT = Ua_w.T, waT = Wa_w.T, va, comb = Wa_b + Ua_b). This removes the
on-device SWDGE cast + xbar transpose chain for weights that serialized
~120us of startup in front of the first uk matmul.

Per-core dataflow (all matmuls bf16 with fp32 PSUM accumulation):
  - uaT/waT strips are direct-loaded [128, strip, h] on the scalar queue.
  - keys strips [128, 2H] are cast-loaded f32->bf16 on gpsimd (kept in
    SBUF for the context matmul), stored to a DRAM scratch (also gpsimd,
    loads batched before stores so transfers overlap), and read back
    transposed ([d=128, s=LG*schunk] tiles) via the DMA xbar on the sync
    queue. kT load-groups are issued one slot ahead of use with a
    two-group buffer pool so the sync queue stays a chunk ahead of PE.
  - ukT tiles [h=128, s=512] accumulate in PSUM; ScalarE applies
    tanh(. + bias[h]) where bias = wq[b] + Wa_b + Ua_b folded per-partition.
  - scores via PE with Va columns as the 1-wide stationary operand.
  - softmax without max-subtraction (scores are O(1)); exp on ScalarE with
    free-dim accumulate for the denominator.
  - context via PE with normalized-late weights columns (PE-transposed
    score chunks) against the cached bf16 keys strips.
"""

import numpy as np

B, TQ, S, H = 32, 4, 2048, 1024
D2 = 2 * H
NCORES = 8
BPC = B // NCORES  # batches per core
NF8 = 4       # leading d-strips done in fp8 e4m3 (DoubleRow, 2x PE rate)
UA_SCALE = 64.0  # Ua pre-scale so fp8 quantization stays in normal range

_CACHE = {}


def _build(s=S, h=H, bpc=BPC, schunk=512, nf8=NF8):
    """Build the per-core Bass module. Parameterized so a scaled-down config
    can run in CoreSim; the shipped kernel uses the defaults."""
    from contextlib import ExitStack

    import concourse.bacc as bacc
    import concourse.mybir as mybir
    import concourse.tile as tile
    from concourse.masks import make_identity

    fp32 = mybir.dt.float32
    bf16 = mybir.dt.bfloat16
    fp8 = mybir.dt.float8e4
    AF = mybir.ActivationFunctionType
    d2 = 2 * h
    SD = d2 // 128        # contraction strips for uk (d on partitions)
    SM = h // 128         # h tiles (uk output partitions / Va strips)
    SJ = h // 128         # contraction strips for wq
    NCH = s // schunk     # score chunks per batch
    SPC = schunk // 128   # keys strips per chunk
    NDC = max(1, d2 // 512)  # context output chunks
    NST = s // 128        # keys strips per batch
    LG = 2 if NCH % 2 == 0 else 1  # chunks per kT load-group
    NPAIR = nf8 // 2      # fp8 DoubleRow d-strip pairs (2x PE rate)
    assert nf8 % 2 == 0 and nf8 <= SD

    nc = bacc.Bacc(
        "TRN2", target_bir_lowering=False, enable_partition_id=False
    )

    q_in = nc.dram_tensor("q", [bpc, h], fp32, kind="ExternalInput").ap()
    keys_in = nc.dram_tensor("keys", [s, bpc, d2], fp32, kind="ExternalInput").ap()
    uaT_in = nc.dram_tensor("uaT", [d2, h], bf16, kind="ExternalInput").ap()
    waT_in = nc.dram_tensor("waT", [h, h], bf16, kind="ExternalInput").ap()
    ua8_in = nc.dram_tensor("uaT8", [nf8 * 128, h], fp8, kind="ExternalInput").ap()
    va_in = nc.dram_tensor("va", [1, h], bf16, kind="ExternalInput").ap()
    comb_in = nc.dram_tensor("comb", [1, h], bf16, kind="ExternalInput").ap()
    ctx_out = nc.dram_tensor("ctx", [bpc, d2], fp32, kind="ExternalOutput").ap()
    w_out = nc.dram_tensor("wts", [bpc, s], fp32, kind="ExternalOutput").ap()

    with tile.TileContext(nc) as tc:
        with ExitStack() as ctx:
            consts = ctx.enter_context(tc.tile_pool(name="consts", bufs=1))
            dram_kn = ctx.enter_context(
                tc.tile_pool(name="dram_kn", bufs=bpc, space="DRAM")
            )
            kcache = ctx.enter_context(
                tc.tile_pool(name="kcache", bufs=3 * SPC + 2)
            )
            ktp = ctx.enter_context(
                tc.tile_pool(name="ktp", bufs=2 * (SD - nf8))
            )
            f8p = ctx.enter_context(
                tc.tile_pool(name="f8p", bufs=2 * max(NPAIR, 1))
            )
            tp = ctx.enter_context(tc.tile_pool(name="tp", bufs=SM + 1))
            rows = ctx.enter_context(tc.tile_pool(name="rows", bufs=2))
            acc1 = ctx.enter_context(tc.tile_pool(name="acc1", bufs=1))
            ps_setup = ctx.enter_context(
                tc.tile_pool(name="ps_setup", bufs=1, space="PSUM")
            )
            ps_uk = ctx.enter_context(tc.tile_pool(name="ps_uk", bufs=4, space="PSUM"))
            ps_sc = ctx.enter_context(tc.tile_pool(name="ps_sc", bufs=1, space="PSUM"))
            ps_cx = ctx.enter_context(tc.tile_pool(name="ps_cx", bufs=2, space="PSUM"))

            # ---------------- one-time setup ----------------
            ident = consts.tile([128, 128], bf16)
            make_identity(nc, ident)
            # only the [1,1] corner is ever used (pscT row transposes)
            ident_f32 = consts.tile([1, 1], fp32)
            nc.vector.memset(ident_f32, 1.0)

            # q cast-load first on gpsimd (gates qT -> bias_cols).
            q_bf = consts.tile([bpc, h], bf16)
            nc.gpsimd.dma_start(out=q_bf, in_=q_in)

            # Small bf16 vectors + transposed weights direct loads (scalar
            # queue; sync is reserved for the keys xbar transposes).
            va_bf = consts.tile([1, h], bf16)
            nc.scalar.dma_start(out=va_bf, in_=va_in)
            comb_bf = consts.tile([1, h], bf16)
            nc.scalar.dma_start(out=comb_bf, in_=comb_in)
            ones_bf = consts.tile([1, bpc], bf16)
            nc.vector.memset(ones_bf, 1.0)

            # Weight loads all ride the sync queue (idle until the first
            # xbar group; scalar stays activation-only). Order = need-time:
            # ua8 gates the first DR matmuls, waT gates bias_cols -> first
            # tanh, uaT's strips gate the bf16 tail of the first m-block.
            ua8 = consts.tile([128, NPAIR, 2, h], fp8)
            for pr in range(NPAIR):
                for t in range(2):
                    nc.sync.dma_start(
                        out=ua8[:, pr, t, :],
                        in_=ua8_in[(2 * pr + t) * 128 : (2 * pr + t + 1) * 128, :],
                    )
            waT = consts.tile([128, SJ, h], bf16)
            for j in range(SJ):
                nc.sync.dma_start(
                    out=waT[:, j, :], in_=waT_in[j * 128 : (j + 1) * 128, :]
                )
            # only the strips the bf16 path still needs (d >= nf8)
            uaT = consts.tile([128, SD - nf8, h], bf16)
            for d in range(nf8, SD):
                nc.sync.dma_start(
                    out=uaT[:, d - nf8, :],
                    in_=uaT_in[d * 128 : (d + 1) * 128, :],
                )

            # qT strips [j=128, bpc] via PE transpose of q_bf
            qT = consts.tile([128, SJ, bpc], bf16)
            for j in range(SJ):
                ptr = ps_setup.tile([128, bpc], bf16, tag="setup")
                nc.tensor.transpose(
                    out=ptr,
                    in_=q_bf[:, j * 128 : (j + 1) * 128],
                    identity=ident[:bpc, :bpc],
                )
                nc.vector.tensor_copy(out=qT[:, j, :], in_=ptr)

            # Va columns [h=128, SM] via PE transpose of the bf16 row
            va_cols = consts.tile([128, SM], bf16)
            for m in range(SM):
                vtr = ps_setup.tile([128, 1], bf16, tag="setup")
                nc.tensor.transpose(
                    out=vtr,
                    in_=va_bf[:1, m * 128 : (m + 1) * 128],
                    identity=ident[:1, :1],
                )
                nc.vector.tensor_copy(out=va_cols[:, m : m + 1], in_=vtr)

            # ---------------- keys pipeline helpers ----------------
            # strips for one chunk: cast-loads f32->bf16 into SBUF (reused by
            # the context matmul), then stores to the DRAM scratch in natural
            # layout. All on gpsimd (only SWDGE can cast); loads are batched
            # before stores so the transfers overlap despite the in-order
            # queue (store i only head-of-line-blocks after load i landed).
            def load_strips(knat, b, c, store=True):
                strips = []
                for i in range(SPC):
                    si = c * SPC + i
                    ks = kcache.tile([128, d2], bf16, tag="ks", name=f"ks_{b}_{si}")
                    nc.gpsimd.dma_start(
                        out=ks, in_=keys_in[si * 128 : (si + 1) * 128, b, :]
                    )
                    strips.append(ks)
                if store:
                    # only the bf16-path columns (d >= nf8) round-trip through
                    # DRAM; the fp8 strips are PE-transposed from SBUF
                    for i in range(SPC):
                        si = c * SPC + i
                        nc.gpsimd.dma_start(
                            out=knat[si * 128 : (si + 1) * 128, :],
                            in_=strips[i][:, nf8 * 128 :],
                        )
                return strips

            def load_kts(knat, b, g):
                # one transposed tile per d covering LG chunks of s: the
                # ~1.3us fixed cost per xbar instruction serializes on the
                # Sync queue, so fewer/bigger transposes keep PE fed
                # bf16 strips only (d >= nf8, index shifted by nf8).
                # All on the sync queue: DMA on the scalar queue races
                # with the activation stream (observed corruption whenever
                # bulk DMA shares the Activation engine's queue mid-kernel)
                kts = []
                for d in range(nf8, SD):
                    kt = ktp.tile(
                        [128, LG * schunk], bf16, tag="kt", name=f"kt_{b}_{g}_{d}"
                    )
                    nc.sync.dma_start(
                        out=kt,
                        in_=knat[
                            g * LG * schunk : (g + 1) * LG * schunk,
                            (d - nf8) * 128 : (d - nf8 + 1) * 128,
                        ],
                        transpose=True,
                    )
                    kts.append(kt)
                return kts

            def pe_f8(chunk_strips, tagname):
                # fp8 kT pair-tiles [128, 2, s] for the DoubleRow 2x matmul
                # path, built by PE transposes straight from the SBUF strips
                # (no DRAM round trip, no wait on the xbar queue) and packed
                # by a single casting vector copy per k-tile
                f8ts = []
                for pr in range(NPAIR):
                    f8t = f8p.tile(
                        [128, 2, LG * schunk], fp8, tag="f8",
                        name=f"f8_{tagname}_{pr}",
                    )
                    for t in range(2):
                        d = 2 * pr + t
                        ptr = ps_uk.tile([128, 2 * SPC * 128], bf16, tag="puk")
                        for cc in range(len(chunk_strips)):
                            for i in range(SPC):
                                nc.tensor.transpose(
                                    out=ptr[
                                        :,
                                        (cc * SPC + i) * 128 : (cc * SPC + i + 1)
                                        * 128,
                                    ],
                                    in_=chunk_strips[cc][i][
                                        :, d * 128 : (d + 1) * 128
                                    ],
                                    identity=ident,
                                )
                        nc.vector.tensor_copy(
                            out=f8t[:, t, :], in_=ptr[:, : LG * schunk]
                        )
                    f8ts.append(f8t)
                return f8ts

            # ---------------- main loop over batches ----------------
            # strips are prefetched PF slots ahead; each kT load-group is
            # issued one slot before its first use so the 16 xbar ops run
            # during the previous chunk's compute.
            # first kT group is built by PE transposes from the SBUF strips
            # (no DRAM round trip): strips land ~12us in, so the first uk
            # matmul can start ~35us instead of waiting ~120us for the xbar
            # chain to clear the startup DMA burst.
            def pe_kts(chunk_strips):
                # chunk_strips: list of LG lists of SPC strips; bf16 path
                # tiles only (the fp8 strips get their own pe_f8 tiles)
                kts = []
                for d in range(nf8, SD):
                    kt = ktp.tile(
                        [128, LG * schunk], bf16, tag="kt", name=f"kt_pe_{d}"
                    )
                    # share the uk PSUM ring: same tag and byte footprint
                    # ([128, 2*SPC*128] bf16 == [128, SPC*128] fp32), so the
                    # transposes triple-buffer without extra banks
                    ptr = ps_uk.tile([128, 2 * SPC * 128], bf16, tag="puk")
                    for cc in range(LG):
                        for i in range(SPC):
                            nc.tensor.transpose(
                                out=ptr[
                                    :,
                                    (cc * SPC + i) * 128 : (cc * SPC + i + 1) * 128,
                                ],
                                in_=chunk_strips[cc][i][
                                    :, d * 128 : (d + 1) * 128
                                ],
                                identity=ident,
                            )
                    nc.vector.tensor_copy(
                        out=kt, in_=ptr[:, : LG * schunk]
                    )
                    kts.append(kt)
                return kts

            seq = [(b, c) for b in range(bpc) for c in range(NCH)]
            PF = 2 if NCH > 1 else 1
            knats = {}
            pending = {}
            pending_kts = {}

            pending_f8 = {}

            knats[0] = dram_kn.tile(
                [s, d2 - nf8 * 128], bf16, tag="knat", name="knat_b0"
            )
            pending[(0, 0)] = load_strips(knats[0], 0, 0, store=False)
            if LG > 1:
                pending[(0, 1)] = load_strips(knats[0], 0, 1, store=False)
                pending_kts[(0, 0)] = pe_kts([pending[(0, 0)], pending[(0, 1)]])
            else:
                pending_kts[(0, 0)] = pe_kts([pending[(0, 0)]])
                if NCH > 1:
                    pending[(0, 1)] = load_strips(knats[0], 0, 1)
            if NPAIR:
                prime_strips = [pending[(0, 0)]]
                if LG > 1:
                    prime_strips.append(pending[(0, 1)])
                pending_f8[(0, 0)] = pe_f8(prime_strips, "g00")

            # bias_cols[:, m, b] = (Wa q_b)[128m:128m+128] + Wa_b + Ua_b (fp32)
            # Emitted AFTER the pe_kts transposes: waT loads pace in behind
            # the strip burst, and these matmuls must not stall the in-order
            # PE queue ahead of the first-group transposes. They complete
            # well before the first tanh needs the bias.
            bias_cols = consts.tile([128, SM, bpc], fp32)
            for m in range(SM):
                pw = ps_setup.tile([128, bpc], fp32, tag="setup")
                for j in range(SJ):
                    nc.tensor.matmul(
                        out=pw,
                        lhsT=waT[:, j, m * 128 : (m + 1) * 128],
                        rhs=qT[:, j, :],
                        start=(j == 0),
                        stop=False,
                    )
                nc.tensor.matmul(
                    out=pw,
                    lhsT=comb_bf[:1, m * 128 : (m + 1) * 128],
                    rhs=ones_bf,
                    start=False,
                    stop=True,
                )
                nc.vector.tensor_copy(out=bias_cols[:, m, :], in_=pw)

            kts_group = None
            for b in range(bpc):
                exp_row = rows.tile([1, s], fp32, tag="exp_row")
                tparts = rows.tile([1, NCH], fp32, tag="tparts")
                ecols = rows.tile([128, NST], bf16, tag="ecols")
                ctx_acc = acc1.tile([1, d2], fp32, tag="ctx_acc")
                for c in range(NCH):
                    f8_todo = None
                    strips = pending.pop((b, c))
                    if c % LG == 0:
                        kts_group = pending_kts.pop((b, c // LG))
                        f8_group = (
                            pending_f8.pop((b, c // LG)) if NPAIR else None
                        )
                    sub = c % LG
                    pos = b * NCH + c
                    # prefetch strips PF slots ahead
                    if pos + PF < len(seq):
                        nb, nxc = seq[pos + PF]
                        if nb not in knats:
                            knats[nb] = dram_kn.tile(
                                [s, d2 - nf8 * 128], bf16, tag="knat",
                                name=f"knat_b{nb}",
                            )
                        pending[(nb, nxc)] = load_strips(knats[nb], nb, nxc)
                    # issue the NEXT slot's kT group (if one starts there)
                    if pos + 1 < len(seq):
                        nb, nxc = seq[pos + 1]
                        if nxc % LG == 0:
                            pending_kts[(nb, nxc // LG)] = load_kts(
                                knats[nb], nb, nxc // LG
                            )
                            if NPAIR:
                                # fp8 tiles come from this group's SBUF
                                # strips; built at the END of this slot so
                                # the in-order PE queue is not parked on
                                # the strip loads issued just above
                                nsl = [pending[(nb, nxc)]]
                                if LG > 1:
                                    nsl.append(pending[(nb, nxc + 1)])
                                f8_todo = (
                                    (nb, nxc // LG), nsl,
                                    f"g{nb}_{nxc // LG}",
                                )
                    # ukT tiles + tanh; score matmuls are deferred until all
                    # tanh tiles exist so the in-order PE queue never waits
                    # on the Scalar engine mid-chunk
                    psc = ps_sc.tile([1, schunk], fp32, tag="psc")
                    ts_list = []
                    for m in range(SM):
                        puk = ps_uk.tile([128, schunk], fp32, tag="puk")
                        for pr in range(NPAIR):
                            nc.tensor.matmul(
                                out=puk,
                                lhsT=ua8[:, pr, :, m * 128 : (m + 1) * 128],
                                rhs=f8_group[pr][
                                    :, :, sub * schunk : (sub + 1) * schunk
                                ],
                                start=(pr == 0),
                                stop=False,
                                perf_mode=mybir.MatmulPerfMode.DoubleRow,
                            )
                        for d in range(nf8, SD):
                            nc.tensor.matmul(
                                out=puk,
                                lhsT=uaT[:, d - nf8, m * 128 : (m + 1) * 128],
                                rhs=kts_group[d - nf8][
                                    :, sub * schunk : (sub + 1) * schunk
                                ],
                                start=(d == nf8 and not NPAIR),
                                stop=(d == SD - 1),
                            )
                        t_sb = tp.tile([128, schunk], bf16, tag="t")
                        nc.scalar.activation(
                            out=t_sb,
                            in_=puk,
                            func=AF.Tanh,
                            bias=bias_cols[:, m, b : b + 1],
                            scale=1.0 / UA_SCALE,
                        )
                        ts_list.append(t_sb)
                    for m in range(SM):
                        nc.tensor.matmul(
                            out=psc,
                            lhsT=va_cols[:, m : m + 1],
                            rhs=ts_list[m],
                            start=(m == 0),
                            stop=(m == SM - 1),
                        )
                    # exp row chunk (no max subtraction; scores are O(1)) and
                    # the chunk's softmax partial sum
                    nc.scalar.activation(
                        out=exp_row[:, c * schunk : (c + 1) * schunk],
                        in_=psc,
                        func=AF.Exp,
                        accum_out=tparts[:, c : c + 1],
                    )
                    # transpose this chunk's scores into columns on PE (tiny)
                    # and exp them -> unnormalized weight columns for context
                    scsb = rows.tile([1, schunk], fp32, tag="scsb")
                    nc.vector.tensor_copy(out=scsb, in_=psc)
                    pscT = ps_setup.tile([128, SPC], fp32, tag="setup")
                    for g in range(SPC):
                        nc.tensor.transpose(
                            out=pscT[:, g : g + 1],
                            in_=scsb[:1, g * 128 : (g + 1) * 128],
                            identity=ident_f32[:1, :1],
                        )
                    nc.scalar.activation(
                        out=ecols[:, c * SPC : (c + 1) * SPC],
                        in_=pscT,
                        func=AF.Exp,
                    )
                    # context partial for this chunk's strips (normalized at
                    # the end of the batch): ctx += sum_si e[si] * k[si, :]
                    for jd in range(NDC):
                        pcx = ps_cx.tile([1, min(512, d2)], fp32, tag="pcx")
                        for i in range(SPC):
                            nc.tensor.matmul(
                                out=pcx,
                                lhsT=ecols[:, c * SPC + i : c * SPC + i + 1],
                                rhs=strips[i][:, jd * 512 : (jd + 1) * 512],
                                start=(i == 0),
                                stop=(i == SPC - 1),
                            )
                        if c == 0:
                            nc.vector.tensor_copy(
                                out=ctx_acc[:, jd * 512 : (jd + 1) * 512], in_=pcx
                            )
                        else:
                            nc.vector.tensor_add(
                                out=ctx_acc[:, jd * 512 : (jd + 1) * 512],
                                in0=ctx_acc[:, jd * 512 : (jd + 1) * 512],
                                in1=pcx,
                            )
                    if f8_todo is not None:
                        fkey, fsl, ftag = f8_todo
                        pending_f8[fkey] = pe_f8(fsl, ftag)
                # softmax denominator; normalize weights + context, write out
                tsum = rows.tile([1, 1], fp32, tag="tsum")
                nc.vector.reduce_sum(
                    out=tsum, in_=tparts, axis=mybir.AxisListType.X
                )
                invt = rows.tile([1, 1], fp32, tag="invt")
                nc.vector.reciprocal(out=invt, in_=tsum)
                nc.vector.tensor_scalar_mul(out=exp_row, in0=exp_row, scalar1=invt)
                nc.scalar.dma_start(out=w_out[b : b + 1, :], in_=exp_row)
                nc.vector.tensor_scalar_mul(out=ctx_acc, in0=ctx_acc, scalar1=invt)
                nc.scalar.dma_start(out=ctx_out[b : b + 1, :], in_=ctx_acc)

    nc.compile()
    return nc


def _get_nc():
    if "nc" not in _CACHE:
        _CACHE["nc"] = _build()
    return _CACHE["nc"]


def _prep_weights(Wa_w, Wa_b, Ua_w, Ua_b, Va_w, nf8=NF8):
    """Host-side weight pre-layout: transpose + cast once. uaT is pre-scaled
    by UA_SCALE (exponent-only in bf16, exact) so the fp8 copy of its leading
    strips quantizes in e4m3's normal range; the kernel's tanh un-scales."""
    import ml_dtypes

    bf = ml_dtypes.bfloat16
    f8 = ml_dtypes.float8_e4m3
    h = Wa_w.shape[0]
    uaT_s = np.ascontiguousarray(
        np.asarray(Ua_w, dtype=np.float32).T * UA_SCALE
    )
    return {
        "uaT": uaT_s.astype(bf),
        "uaT8": uaT_s[: nf8 * 128].astype(f8),
        "waT": np.ascontiguousarray(np.asarray(Wa_w, dtype=np.float32).T).astype(bf),
        "va": np.asarray(Va_w, dtype=np.float32).reshape(1, h).astype(bf),
        "comb": (
            np.asarray(Wa_b, dtype=np.float32) + np.asarray(Ua_b, dtype=np.float32)
        ).reshape(1, h).astype(bf),
    }


def _make_in_maps(inputs):
    q_last = np.ascontiguousarray(
        np.asarray(inputs["query"], dtype=np.float32)[:, -1, :]
    )  # [B, H]
    keys = np.asarray(inputs["keys"], dtype=np.float32)  # [S, B, 2H]
    weights = _prep_weights(
        inputs["Wa_w"], inputs["Wa_b"], inputs["Ua_w"], inputs["Ua_b"], inputs["Va_w"]
    )
    in_maps = []
    for c in range(NCORES):
        b0 = c * BPC
        in_maps.append(
            {
                "q": np.ascontiguousarray(q_last[b0 : b0 + BPC]),
                "keys": np.ascontiguousarray(keys[:, b0 : b0 + BPC, :]),
                **weights,
            }
        )
    return in_maps


def run(inputs, trace=False, **kwargs):
    """Run on all 8 cores; returns ((context, weights), BassKernelResults)."""
    from concourse.bass_utils import run_bass_kernel_spmd

    nc = _get_nc()
    in_maps = _make_in_maps(inputs)
    res = run_bass_kernel_spmd(
        nc, in_maps, core_ids=list(range(NCORES)), trace=trace, **kwargs
    )
    context = np.empty((B, 1, D2), dtype=np.float32)
    weights = np.empty((B, 1, S), dtype=np.float32)
    for c in range(NCORES):
        b0 = c * BPC
        context[b0 : b0 + BPC, 0, :] = res.results[c]["ctx"]
        weights[b0 : b0 + BPC, 0, :] = res.results[c]["wts"]
    return (context, weights), res


def kernel(**inputs):
    out, _ = run(inputs)
    return out


# revision 27
# speedup vs baseline: 1.0824x; 1.0824x over previous
"""Bahdanau additive attention kernel for Trainium2 (8 NeuronCores, SPMD).

Problem (hardcoded): B=32, Tq=4, S=2048, H=1024, 2H=2048, fp32 inputs.
  q  = query[:, -1, :]                      [B, H]
  k  = transpose(keys, (1, 0, 2))           [B, S, 2H]
  wq = q @ Wa_w.T + Wa_b                    [B, H]
  uk = k @ Ua_w.T + Ua_b                    [B, S, H]
  sc = tanh(wq[:, None, :] + uk) @ Va_w.T   [B, S]   (+ Va_b, which softmax cancels)
  w  = softmax(sc, axis=-1)                 [B, S]
  ctx = w @ k                               [B, 2H]
  returns (ctx [B,1,2H], w [B,1,S])

Sharding: data-parallel over batch. 8 cores x 4 batches each; weights
replicated; no cross-core communication.

Host prep: weights are pre-transposed and cast to bf16 on the host
(uaT = Ua_w.T, waT = Wa_w.T, va, comb = Wa_b + Ua_b). This removes the
on-device SWDGE cast + xbar transpose chain for weights that serialized
~120us of startup in front of the first uk matmul.

Per-core dataflow (all matmuls bf16 with fp32 PSUM accumulation):
  - uaT/waT strips are direct-loaded [128, strip, h] on the scalar queue.
  - keys strips [128, 2H] are cast-loaded f32->bf16 on gpsimd (kept in
    SBUF for the context matmul), stored to a DRAM scratch (also gpsimd,
    loads batched before stores so transfers overlap), and read back
    transposed ([d=128, s=LG*schunk] tiles) via the DMA xbar on the sync
    queue. kT load-groups are issued one slot ahead of use with a
    two-group buffer pool so the sync queue stays a chunk ahead of PE.
  - ukT tiles [h=128, s=512] accumulate in PSUM; ScalarE applies
    tanh(. + bias[h]) where bias = wq[b] + Wa_b + Ua_b folded per-partition.
  - scores via PE with Va columns as the 1-wide stationary operand.
  - softmax without max-subtraction (scores are O(1)); exp on ScalarE with
    free-dim accumulate for the denominator.
  - context via PE with normalized-late weights columns (PE-transposed
    score chunks) against the cached bf16 keys strips.
"""

import numpy as np

B, TQ, S, H = 32, 4, 2048, 1024
D2 = 2 * H
NCORES = 8
BPC = B // NCORES  # batches per core
NF8 = 4       # leading d-strips done in fp8 e4m3 (DoubleRow, 2x PE rate)
UA_SCALE = 64.0  # Ua pre-scale so fp8 quantization stays in normal range

_CACHE = {}


def _build(s=S, h=H, bpc=BPC, schunk=512, nf8=NF8):
    """Build the per-core Bass module. Parameterized so a scaled-down config
    can run in CoreSim; the shipped kernel uses the defaults."""
    from contextlib import ExitStack

    import concourse.bacc as bacc
    import concourse.mybir as mybir
    import concourse.tile as tile
    from concourse.masks import make_identity

    fp32 = mybir.dt.float32
    bf16 = mybir.dt.bfloat16
    fp8 = mybir.dt.float8e4
    AF = mybir.ActivationFunctionType
    d2 = 2 * h
    SD = d2 // 128        # contraction strips for uk (d on partitions)
    SM = h // 128         # h tiles (uk output partitions / Va strips)
    SJ = h // 128         # contraction strips for wq
    NCH = s // schunk     # score chunks per batch
    SPC = schunk // 128   # keys strips per chunk
    NDC = max(1, d2 // 512)  # context output chunks
    NST = s // 128        # keys strips per batch
    LG = 2 if NCH % 2 == 0 else 1  # chunks per kT load-group
    NPAIR = nf8 // 2      # fp8 DoubleRow d-strip pairs (2x PE rate)
    assert nf8 % 2 == 0 and nf8 <= SD

    nc = bacc.Bacc(
        "TRN2", target_bir_lowering=False, enable_partition_id=False
    )

    q_in = nc.dram_tensor("q", [bpc, h], fp32, kind="ExternalInput").ap()
    keys_in = nc.dram_tensor("keys", [s, bpc, d2], fp32, kind="ExternalInput").ap()
    uaT_in = nc.dram_tensor("uaT", [d2, h], bf16, kind="ExternalInput").ap()
    waT_in = nc.dram_tensor("waT", [h, h], bf16, kind="ExternalInput").ap()
    ua8_in = nc.dram_tensor("uaT8", [nf8 * 128, h], fp8, kind="ExternalInput").ap()
    va_in = nc.dram_tensor("va", [1, h], bf16, kind="ExternalInput").ap()
    comb_in = nc.dram_tensor("comb", [1, h], bf16, kind="ExternalInput").ap()
    ctx_out = nc.dram_tensor("ctx", [bpc, d2], fp32, kind="ExternalOutput").ap()
    w_out = nc.dram_tensor("wts", [bpc, s], fp32, kind="ExternalOutput").ap()

    with tile.TileContext(nc) as tc:
        with ExitStack() as ctx:
            consts = ctx.enter_context(tc.tile_pool(name="consts", bufs=1))
            dram_kn = ctx.enter_context(
                tc.tile_pool(name="dram_kn", bufs=bpc, space="DRAM")
            )
            kcache = ctx.enter_context(
                tc.tile_pool(name="kcache", bufs=3 * SPC + 2)
            )
            ktp = ctx.enter_context(
                tc.tile_pool(name="ktp", bufs=2 * (SD - nf8))
            )
            f8p = ctx.enter_context(
                tc.tile_pool(name="f8p", bufs=2 * max(NPAIR, 1))
            )
            tp = ctx.enter_context(tc.tile_pool(name="tp", bufs=SM + 1))
            rows = ctx.enter_context(tc.tile_pool(name="rows", bufs=2))
            acc1 = ctx.enter_context(tc.tile_pool(name="acc1", bufs=1))
            ps_setup = ctx.enter_context(
                tc.tile_pool(name="ps_setup", bufs=1, space="PSUM")
            )
            ps_uk = ctx.enter_context(tc.tile_pool(name="ps_uk", bufs=3, space="PSUM"))
            ps_sc = ctx.enter_context(tc.tile_pool(name="ps_sc", bufs=2, space="PSUM"))
            ps_cx = ctx.enter_context(tc.tile_pool(name="ps_cx", bufs=2, space="PSUM"))

            # ---------------- one-time setup ----------------
            ident = consts.tile([128, 128], bf16)
            make_identity(nc, ident)
            # only the [1,1] corner is ever used (pscT row transposes)
            ident_f32 = consts.tile([1, 1], fp32)
            nc.vector.memset(ident_f32, 1.0)

            # q cast-load first on gpsimd (gates qT -> bias_cols).
            q_bf = consts.tile([bpc, h], bf16)
            nc.gpsimd.dma_start(out=q_bf, in_=q_in)

            # Small bf16 vectors + transposed weights direct loads (scalar
            # queue; sync is reserved for the keys xbar transposes).
            va_bf = consts.tile([1, h], bf16)
            nc.scalar.dma_start(out=va_bf, in_=va_in)
            comb_bf = consts.tile([1, h], bf16)
            nc.scalar.dma_start(out=comb_bf, in_=comb_in)
            ones_bf = consts.tile([1, bpc], bf16)
            nc.vector.memset(ones_bf, 1.0)

            # uaT[:, d, :] = Ua[:, 128d:128d+128].T  -> [dpart=128, h]
            # uaT rides the sync queue (idle until the first xbar group at
            # ~60us and directly gates the first uk matmuls); waT on scalar.
            # only the strips the bf16 path still needs (d >= nf8)
            uaT = consts.tile([128, SD - nf8, h], bf16)
            for d in range(nf8, SD):
                nc.sync.dma_start(
                    out=uaT[:, d - nf8, :],
                    in_=uaT_in[d * 128 : (d + 1) * 128, :],
                )
            ua8 = consts.tile([128, NPAIR, 2, h], fp8)
            for pr in range(NPAIR):
                for t in range(2):
                    nc.sync.dma_start(
                        out=ua8[:, pr, t, :],
                        in_=ua8_in[(2 * pr + t) * 128 : (2 * pr + t + 1) * 128, :],
                    )
            waT = consts.tile([128, SJ, h], bf16)
            for j in range(SJ):
                nc.scalar.dma_start(
                    out=waT[:, j, :], in_=waT_in[j * 128 : (j + 1) * 128, :]
                )

            # qT strips [j=128, bpc] via PE transpose of q_bf
            qT = consts.tile([128, SJ, bpc], bf16)
            for j in range(SJ):
                ptr = ps_setup.tile([128, bpc], bf16, tag="setup")
                nc.tensor.transpose(
                    out=ptr,
                    in_=q_bf[:, j * 128 : (j + 1) * 128],
                    identity=ident[:bpc, :bpc],
                )
                nc.vector.tensor_copy(out=qT[:, j, :], in_=ptr)

            # Va columns [h=128, SM] via PE transpose of the bf16 row
            va_cols = consts.tile([128, SM], bf16)
            for m in range(SM):
                vtr = ps_setup.tile([128, 1], bf16, tag="setup")
                nc.tensor.transpose(
                    out=vtr,
                    in_=va_bf[:1, m * 128 : (m + 1) * 128],
                    identity=ident[:1, :1],
                )
                nc.vector.tensor_copy(out=va_cols[:, m : m + 1], in_=vtr)

            # ---------------- keys pipeline helpers ----------------
            # strips for one chunk: cast-loads f32->bf16 into SBUF (reused by
            # the context matmul), then stores to the DRAM scratch in natural
            # layout. All on gpsimd (only SWDGE can cast); loads are batched
            # before stores so the transfers overlap despite the in-order
            # queue (store i only head-of-line-blocks after load i landed).
            def load_strips(knat, b, c, store=True):
                strips = []
                for i in range(SPC):
                    si = c * SPC + i
                    ks = kcache.tile([128, d2], bf16, tag="ks", name=f"ks_{b}_{si}")
                    nc.gpsimd.dma_start(
                        out=ks, in_=keys_in[si * 128 : (si + 1) * 128, b, :]
                    )
                    strips.append(ks)
                if store:
                    # only the bf16-path columns (d >= nf8) round-trip through
                    # DRAM; the fp8 strips are PE-transposed from SBUF
                    for i in range(SPC):
                        si = c * SPC + i
                        nc.gpsimd.dma_start(
                            out=knat[si * 128 : (si + 1) * 128, :],
                            in_=strips[i][:, nf8 * 128 :],
                        )
                return strips

            def load_kts(knat, b, g):
                # one transposed tile per d covering LG chunks of s: the
                # ~1.3us fixed cost per xbar instruction serializes on the
                # Sync queue, so fewer/bigger transposes keep PE fed
                # bf16 strips only (d >= nf8, index shifted by nf8).
                # All on the sync queue: DMA on the scalar queue races
                # with the activation stream (observed corruption whenever
                # bulk DMA shares the Activation engine's queue mid-kernel)
                kts = []
                for d in range(nf8, SD):
                    kt = ktp.tile(
                        [128, LG * schunk], bf16, tag="kt", name=f"kt_{b}_{g}_{d}"
                    )
                    nc.sync.dma_start(
                        out=kt,
                        in_=knat[
                            g * LG * schunk : (g + 1) * LG * schunk,
                            (d - nf8) * 128 : (d - nf8 + 1) * 128,
                        ],
                        transpose=True,
                    )
                    kts.append(kt)
                return kts

            def pe_f8(chunk_strips, tagname):
                # fp8 kT pair-tiles [128, 2, s] for the DoubleRow 2x matmul
                # path, built by PE transposes straight from the SBUF strips
                # (no DRAM round trip, no wait on the xbar queue) and packed
                # by a single casting vector copy per k-tile
                f8ts = []
                for pr in range(NPAIR):
                    f8t = f8p.tile(
                        [128, 2, LG * schunk], fp8, tag="f8",
                        name=f"f8_{tagname}_{pr}",
                    )
                    for t in range(2):
                        d = 2 * pr + t
                        ptr = ps_uk.tile([128, 2 * SPC * 128], bf16, tag="puk")
                        for cc in range(len(chunk_strips)):
                            for i in range(SPC):
                                nc.tensor.transpose(
                                    out=ptr[
                                        :,
                                        (cc * SPC + i) * 128 : (cc * SPC + i + 1)
                                        * 128,
                                    ],
                                    in_=chunk_strips[cc][i][
                                        :, d * 128 : (d + 1) * 128
                                    ],
                                    identity=ident,
                                )
                        nc.vector.tensor_copy(
                            out=f8t[:, t, :], in_=ptr[:, : LG * schunk]
                        )
                    f8ts.append(f8t)
                return f8ts

            # ---------------- main loop over batches ----------------
            # strips are prefetched PF slots ahead; each kT load-group is
            # issued one slot before its first use so the 16 xbar ops run
            # during the previous chunk's compute.
            # first kT group is built by PE transposes from the SBUF strips
            # (no DRAM round trip): strips land ~12us in, so the first uk
            # matmul can start ~35us instead of waiting ~120us for the xbar
            # chain to clear the startup DMA burst.
            def pe_kts(chunk_strips):
                # chunk_strips: list of LG lists of SPC strips; bf16 path
                # tiles only (the fp8 strips get their own pe_f8 tiles)
                kts = []
                for d in range(nf8, SD):
                    kt = ktp.tile(
                        [128, LG * schunk], bf16, tag="kt", name=f"kt_pe_{d}"
                    )
                    # share the uk PSUM ring: same tag and byte footprint
                    # ([128, 2*SPC*128] bf16 == [128, SPC*128] fp32), so the
                    # transposes triple-buffer without extra banks
                    ptr = ps_uk.tile([128, 2 * SPC * 128], bf16, tag="puk")
                    for cc in range(LG):
                        for i in range(SPC):
                            nc.tensor.transpose(
                                out=ptr[
                                    :,
                                    (cc * SPC + i) * 128 : (cc * SPC + i + 1) * 128,
                                ],
                                in_=chunk_strips[cc][i][
                                    :, d * 128 : (d + 1) * 128
                                ],
                                identity=ident,
                            )
                    nc.vector.tensor_copy(
                        out=kt, in_=ptr[:, : LG * schunk]
                    )
                    kts.append(kt)
                return kts

            seq = [(b, c) for b in range(bpc) for c in range(NCH)]
            PF = 2 if NCH > 1 else 1
            knats = {}
            pending = {}
            pending_kts = {}

            pending_f8 = {}

            knats[0] = dram_kn.tile(
                [s, d2 - nf8 * 128], bf16, tag="knat", name="knat_b0"
            )
            pending[(0, 0)] = load_strips(knats[0], 0, 0, store=False)
            if LG > 1:
                pending[(0, 1)] = load_strips(knats[0], 0, 1, store=False)
                pending_kts[(0, 0)] = pe_kts([pending[(0, 0)], pending[(0, 1)]])
            else:
                pending_kts[(0, 0)] = pe_kts([pending[(0, 0)]])
                if NCH > 1:
                    pending[(0, 1)] = load_strips(knats[0], 0, 1)
            if NPAIR:
                prime_strips = [pending[(0, 0)]]
                if LG > 1:
                    prime_strips.append(pending[(0, 1)])
                pending_f8[(0, 0)] = pe_f8(prime_strips, "g00")

            # bias_cols[:, m, b] = (Wa q_b)[128m:128m+128] + Wa_b + Ua_b (fp32)
            # Emitted AFTER the pe_kts transposes: waT loads pace in behind
            # the strip burst, and these matmuls must not stall the in-order
            # PE queue ahead of the first-group transposes. They complete
            # well before the first tanh needs the bias.
            bias_cols = consts.tile([128, SM, bpc], fp32)
            for m in range(SM):
                pw = ps_setup.tile([128, bpc], fp32, tag="setup")
                for j in range(SJ):
                    nc.tensor.matmul(
                        out=pw,
                        lhsT=waT[:, j, m * 128 : (m + 1) * 128],
                        rhs=qT[:, j, :],
                        start=(j == 0),
                        stop=False,
                    )
                nc.tensor.matmul(
                    out=pw,
                    lhsT=comb_bf[:1, m * 128 : (m + 1) * 128],
                    rhs=ones_bf,
                    start=False,
                    stop=True,
                )
                nc.vector.tensor_copy(out=bias_cols[:, m, :], in_=pw)

            kts_group = None
            for b in range(bpc):
                exp_row = rows.tile([1, s], fp32, tag="exp_row")
                tparts = rows.tile([1, NCH], fp32, tag="tparts")
                ecols = rows.tile([128, NST], bf16, tag="ecols")
                ctx_acc = acc1.tile([1, d2], fp32, tag="ctx_acc")
                for c in range(NCH):
                    f8_todo = None
                    strips = pending.pop((b, c))
                    if c % LG == 0:
                        kts_group = pending_kts.pop((b, c // LG))
                        f8_group = (
                            pending_f8.pop((b, c // LG)) if NPAIR else None
                        )
                    sub = c % LG
                    pos = b * NCH + c
                    # prefetch strips PF slots ahead
                    if pos + PF < len(seq):
                        nb, nxc = seq[pos + PF]
                        if nb not in knats:
                            knats[nb] = dram_kn.tile(
                                [s, d2 - nf8 * 128], bf16, tag="knat",
                                name=f"knat_b{nb}",
                            )
                        pending[(nb, nxc)] = load_strips(knats[nb], nb, nxc)
                    # issue the NEXT slot's kT group (if one starts there)
                    if pos + 1 < len(seq):
                        nb, nxc = seq[pos + 1]
                        if nxc % LG == 0:
                            pending_kts[(nb, nxc // LG)] = load_kts(
                                knats[nb], nb, nxc // LG
                            )
                            if NPAIR:
                                # fp8 tiles come from this group's SBUF
                                # strips; built at the END of this slot so
                                # the in-order PE queue is not parked on
                                # the strip loads issued just above
                                nsl = [pending[(nb, nxc)]]
                                if LG > 1:
                                    nsl.append(pending[(nb, nxc + 1)])
                                f8_todo = (
                                    (nb, nxc // LG), nsl,
                                    f"g{nb}_{nxc // LG}",
                                )
                    # ukT tiles + tanh; score matmuls are deferred until all
                    # tanh tiles exist so the in-order PE queue never waits
                    # on the Scalar engine mid-chunk
                    psc = ps_sc.tile([1, schunk], fp32, tag="psc")
                    ts_list = []
                    for m in range(SM):
                        puk = ps_uk.tile([128, schunk], fp32, tag="puk")
                        for pr in range(NPAIR):
                            nc.tensor.matmul(
                                out=puk,
                                lhsT=ua8[:, pr, :, m * 128 : (m + 1) * 128],
                                rhs=f8_group[pr][
                                    :, :, sub * schunk : (sub + 1) * schunk
                                ],
                                start=(pr == 0),
                                stop=False,
                                perf_mode=mybir.MatmulPerfMode.DoubleRow,
                            )
                        for d in range(nf8, SD):
                            nc.tensor.matmul(
                                out=puk,
                                lhsT=uaT[:, d - nf8, m * 128 : (m + 1) * 128],
                                rhs=kts_group[d - nf8][
                                    :, sub * schunk : (sub + 1) * schunk
                                ],
                                start=(d == nf8 and not NPAIR),
                                stop=(d == SD - 1),
                            )
                        t_sb = tp.tile([128, schunk], bf16, tag="t")
                        nc.scalar.activation(
                            out=t_sb,
                            in_=puk,
                            func=AF.Tanh,
                            bias=bias_cols[:, m, b : b + 1],
                            scale=1.0 / UA_SCALE,
                        )
                        ts_list.append(t_sb)
                    for m in range(SM):
                        nc.tensor.matmul(
                            out=psc,
                            lhsT=va_cols[:, m : m + 1],
                            rhs=ts_list[m],
                            start=(m == 0),
                            stop=(m == SM - 1),
                        )
                    # exp row chunk (no max subtraction; scores are O(1)) and
                    # the chunk's softmax partial sum
                    nc.scalar.activation(
                        out=exp_row[:, c * schunk : (c + 1) * schunk],
                        in_=psc,
                        func=AF.Exp,
                        accum_out=tparts[:, c : c + 1],
                    )
                    # transpose this chunk's scores into columns on PE (tiny)
                    # and exp them -> unnormalized weight columns for context
                    scsb = rows.tile([1, schunk], fp32, tag="scsb")
                    nc.vector.tensor_copy(out=scsb, in_=psc)
                    pscT = ps_setup.tile([128, SPC], fp32, tag="setup")
                    for g in range(SPC):
                        nc.tensor.transpose(
                            out=pscT[:, g : g + 1],
                            in_=scsb[:1, g * 128 : (g + 1) * 128],
                            identity=ident_f32[:1, :1],
                        )
                    nc.scalar.activation(
                        out=ecols[:, c * SPC : (c + 1) * SPC],
                        in_=pscT,
                        func=AF.Exp,
                    )
                    # context partial for this chunk's strips (normalized at
                    # the end of the batch): ctx += sum_si e[si] * k[si, :]
                    for jd in range(NDC):
                        pcx = ps_cx.tile([1, min(512, d2)], fp32, tag="pcx")
                        for i in range(SPC):
                            nc.tensor.matmul(
                                out=pcx,
                                lhsT=ecols[:, c * SPC + i : c * SPC + i + 1],
                                rhs=strips[i][:, jd * 512 : (jd + 1) * 512],
                                start=(i == 0),
                                stop=(i == SPC - 1),
                            )
                        if c == 0:
                            nc.vector.tensor_copy(
                                out=ctx_acc[:, jd * 512 : (jd + 1) * 512], in_=pcx
                            )
                        else:
                            nc.vector.tensor_add(
                                out=ctx_acc[:, jd * 512 : (jd + 1) * 512],
                                in0=ctx_acc[:, jd * 512 : (jd + 1) * 512],
                                in1=pcx,
                            )
                    if f8_todo is not None:
                        fkey, fsl, ftag = f8_todo
                        pending_f8[fkey] = pe_f8(fsl, ftag)
                # softmax denominator; normalize weights + context, write out
                tsum = rows.tile([1, 1], fp32, tag="tsum")
                nc.vector.reduce_sum(
                    out=tsum, in_=tparts, axis=mybir.AxisListType.X
                )
                invt = rows.tile([1, 1], fp32, tag="invt")
                nc.vector.reciprocal(out=invt, in_=tsum)
                nc.vector.tensor_scalar_mul(out=exp_row, in0=exp_row, scalar1=invt)
                nc.scalar.dma_start(out=w_out[b : b + 1, :], in_=exp_row)
                nc.vector.tensor_scalar_mul(out=ctx_acc, in0=ctx_acc, scalar1=invt)
                nc.scalar.dma_start(out=ctx_out[b : b + 1, :], in_=ctx_acc)

    nc.compile()
    return nc


def _get_nc():
    if "nc" not in _CACHE:
        _CACHE["nc"] = _build()
    return _CACHE["nc"]


def _prep_weights(Wa_w, Wa_b, Ua_w, Ua_b, Va_w, nf8=NF8):
    """Host-side weight pre-layout: transpose + cast once. uaT is pre-scaled
    by UA_SCALE (exponent-only in bf16, exact) so the fp8 copy of its leading
    strips quantizes in e4m3's normal range; the kernel's tanh un-scales."""
    import ml_dtypes

    bf = ml_dtypes.bfloat16
    f8 = ml_dtypes.float8_e4m3
    h = Wa_w.shape[0]
    uaT_s = np.ascontiguousarray(
        np.asarray(Ua_w, dtype=np.float32).T * UA_SCALE
    )
    return {
        "uaT": uaT_s.astype(bf),
        "uaT8": uaT_s[: nf8 * 128].astype(f8),
        "waT": np.ascontiguousarray(np.asarray(Wa_w, dtype=np.float32).T).astype(bf),
        "va": np.asarray(Va_w, dtype=np.float32).reshape(1, h).astype(bf),
        "comb": (
            np.asarray(Wa_b, dtype=np.float32) + np.asarray(Ua_b, dtype=np.float32)
        ).reshape(1, h).astype(bf),
    }


def _make_in_maps(inputs):
    q_last = np.ascontiguousarray(
        np.asarray(inputs["query"], dtype=np.float32)[:, -1, :]
    )  # [B, H]
    keys = np.asarray(inputs["keys"], dtype=np.float32)  # [S, B, 2H]
    weights = _prep_weights(
        inputs["Wa_w"], inputs["Wa_b"], inputs["Ua_w"], inputs["Ua_b"], inputs["Va_w"]
    )
    in_maps = []
    for c in range(NCORES):
        b0 = c * BPC
        in_maps.append(
            {
                "q": np.ascontiguousarray(q_last[b0 : b0 + BPC]),
                "keys": np.ascontiguousarray(keys[:, b0 : b0 + BPC, :]),
                **weights,
            }
        )
    return in_maps


def run(inputs, trace=False, **kwargs):
    """Run on all 8 cores; returns ((context, weights), BassKernelResults)."""
    from concourse.bass_utils import run_bass_kernel_spmd

    nc = _get_nc()
    in_maps = _make_in_maps(inputs)
    res = run_bass_kernel_spmd(
        nc, in_maps, core_ids=list(range(NCORES)), trace=trace, **kwargs
    )
    context = np.empty((B, 1, D2), dtype=np.float32)
    weights = np.empty((B, 1, S), dtype=np.float32)
    for c in range(NCORES):
        b0 = c * BPC
        context[b0 : b0 + BPC, 0, :] = res.results[c]["ctx"]
        weights[b0 : b0 + BPC, 0, :] = res.results[c]["wts"]
    return (context, weights), res


def kernel(**inputs):
    out, _ = run(inputs)
    return out


# revision 28
# speedup vs baseline: 1.2078x; 1.1159x over previous
"""Bahdanau additive attention kernel for Trainium2 (8 NeuronCores, SPMD).

Problem (hardcoded): B=32, Tq=4, S=2048, H=1024, 2H=2048, fp32 inputs.
  q  = query[:, -1, :]                      [B, H]
  k  = transpose(keys, (1, 0, 2))           [B, S, 2H]
  wq = q @ Wa_w.T + Wa_b                    [B, H]
  uk = k @ Ua_w.T + Ua_b                    [B, S, H]
  sc = tanh(wq[:, None, :] + uk) @ Va_w.T   [B, S]   (+ Va_b, which softmax cancels)
  w  = softmax(sc, axis=-1)                 [B, S]
  ctx = w @ k                               [B, 2H]
  returns (ctx [B,1,2H], w [B,1,S])

Sharding: data-parallel over batch. 8 cores x 4 batches each; weights
replicated; no cross-core communication.

Host prep: weights are pre-transposed and cast to bf16 on the host
(uaT = Ua_w.T, waT = Wa_w.T, va, comb = Wa_b + Ua_b). This removes the
on-device SWDGE cast + xbar transpose chain for weights that serialized
~120us of startup in front of the first uk matmul.

Per-core dataflow (all matmuls bf16 with fp32 PSUM accumulation):
  - uaT/waT strips are direct-loaded [128, strip, h] on the scalar queue.
  - keys strips [128, 2H] are cast-loaded f32->bf16 on gpsimd (kept in
    SBUF for the context matmul), stored to a DRAM scratch (also gpsimd,
    loads batched before stores so transfers overlap), and read back
    transposed ([d=128, s=LG*schunk] tiles) via the DMA xbar on the sync
    queue. kT load-groups are issued one slot ahead of use with a
    two-group buffer pool so the sync queue stays a chunk ahead of PE.
  - ukT tiles [h=128, s=512] accumulate in PSUM; ScalarE applies
    tanh(. + bias[h]) where bias = wq[b] + Wa_b + Ua_b folded per-partition.
  - scores via PE with Va columns as the 1-wide stationary operand.
  - softmax without max-subtraction (scores are O(1)); exp on ScalarE with
    free-dim accumulate for the denominator.
  - context via PE with normalized-late weights columns (PE-transposed
    score chunks) against the cached bf16 keys strips.
"""

import numpy as np

B, TQ, S, H = 32, 4, 2048, 1024
D2 = 2 * H
NCORES = 8
BPC = B // NCORES  # batches per core
NF8 = 4       # leading d-strips done in fp8 e4m3 (DoubleRow, 2x PE rate)
UA_SCALE = 64.0  # Ua pre-scale so fp8 quantization stays in normal range

_CACHE = {}


def _build(s=S, h=H, bpc=BPC, schunk=512, nf8=NF8):
    """Build the per-core Bass module. Parameterized so a scaled-down config
    can run in CoreSim; the shipped kernel uses the defaults."""
    from contextlib import ExitStack

    import concourse.bacc as bacc
    import concourse.mybir as mybir
    import concourse.tile as tile
    from concourse.masks import make_identity

    fp32 = mybir.dt.float32
    bf16 = mybir.dt.bfloat16
    fp8 = mybir.dt.float8e4
    AF = mybir.ActivationFunctionType
    d2 = 2 * h
    SD = d2 // 128        # contraction strips for uk (d on partitions)
    SM = h // 128         # h tiles (uk output partitions / Va strips)
    SJ = h // 128         # contraction strips for wq
    NCH = s // schunk     # score chunks per batch
    SPC = schunk // 128   # keys strips per chunk
    NDC = max(1, d2 // 512)  # context output chunks
    NST = s // 128        # keys strips per batch
    LG = 2 if NCH % 2 == 0 else 1  # chunks per kT load-group
    NPAIR = nf8 // 2      # fp8 DoubleRow d-strip pairs (2x PE rate)
    assert nf8 % 2 == 0 and nf8 <= SD

    nc = bacc.Bacc(
        "TRN2", target_bir_lowering=False, enable_partition_id=False
    )

    q_in = nc.dram_tensor("q", [bpc, h], fp32, kind="ExternalInput").ap()
    keys_in = nc.dram_tensor("keys", [s, bpc, d2], fp32, kind="ExternalInput").ap()
    uaT_in = nc.dram_tensor("uaT", [d2, h], bf16, kind="ExternalInput").ap()
    waT_in = nc.dram_tensor("waT", [h, h], bf16, kind="ExternalInput").ap()
    ua8_in = nc.dram_tensor("uaT8", [nf8 * 128, h], fp8, kind="ExternalInput").ap()
    va_in = nc.dram_tensor("va", [1, h], bf16, kind="ExternalInput").ap()
    comb_in = nc.dram_tensor("comb", [1, h], bf16, kind="ExternalInput").ap()
    ctx_out = nc.dram_tensor("ctx", [bpc, d2], fp32, kind="ExternalOutput").ap()
    w_out = nc.dram_tensor("wts", [bpc, s], fp32, kind="ExternalOutput").ap()

    with tile.TileContext(nc) as tc:
        with ExitStack() as ctx:
            consts = ctx.enter_context(tc.tile_pool(name="consts", bufs=1))
            dram_kn = ctx.enter_context(
                tc.tile_pool(name="dram_kn", bufs=bpc, space="DRAM")
            )
            kcache = ctx.enter_context(
                tc.tile_pool(name="kcache", bufs=4 * SPC)
            )
            ktp = ctx.enter_context(
                tc.tile_pool(name="ktp", bufs=2 * (SD - nf8))
            )
            f8p = ctx.enter_context(
                tc.tile_pool(name="f8p", bufs=2 * max(NPAIR, 1))
            )
            tp = ctx.enter_context(tc.tile_pool(name="tp", bufs=SM + 1))
            rows = ctx.enter_context(tc.tile_pool(name="rows", bufs=1))
            acc1 = ctx.enter_context(tc.tile_pool(name="acc1", bufs=1))
            ps_setup = ctx.enter_context(
                tc.tile_pool(name="ps_setup", bufs=1, space="PSUM")
            )
            ps_uk = ctx.enter_context(tc.tile_pool(name="ps_uk", bufs=3, space="PSUM"))
            ps_sc = ctx.enter_context(tc.tile_pool(name="ps_sc", bufs=2, space="PSUM"))
            ps_cx = ctx.enter_context(tc.tile_pool(name="ps_cx", bufs=2, space="PSUM"))

            # ---------------- one-time setup ----------------
            ident = consts.tile([128, 128], bf16)
            make_identity(nc, ident)
            # only the [1,1] corner is ever used (pscT row transposes)
            ident_f32 = consts.tile([1, 1], fp32)
            nc.vector.memset(ident_f32, 1.0)

            # q cast-load first on gpsimd (gates qT -> bias_cols).
            q_bf = consts.tile([bpc, h], bf16)
            nc.gpsimd.dma_start(out=q_bf, in_=q_in)

            # Small bf16 vectors + transposed weights direct loads (scalar
            # queue; sync is reserved for the keys xbar transposes).
            va_bf = consts.tile([1, h], bf16)
            nc.scalar.dma_start(out=va_bf, in_=va_in)
            comb_bf = consts.tile([1, h], bf16)
            nc.scalar.dma_start(out=comb_bf, in_=comb_in)
            ones_bf = consts.tile([1, bpc], bf16)
            nc.vector.memset(ones_bf, 1.0)

            # uaT[:, d, :] = Ua[:, 128d:128d+128].T  -> [dpart=128, h]
            # uaT rides the sync queue (idle until the first xbar group at
            # ~60us and directly gates the first uk matmuls); waT on scalar.
            # only the strips the bf16 path still needs (d >= nf8)
            uaT = consts.tile([128, SD - nf8, h], bf16)
            for d in range(nf8, SD):
                nc.sync.dma_start(
                    out=uaT[:, d - nf8, :],
                    in_=uaT_in[d * 128 : (d + 1) * 128, :],
                )
            ua8 = consts.tile([128, NPAIR, 2, h], fp8)
            for pr in range(NPAIR):
                for t in range(2):
                    nc.sync.dma_start(
                        out=ua8[:, pr, t, :],
                        in_=ua8_in[(2 * pr + t) * 128 : (2 * pr + t + 1) * 128, :],
                    )
            waT = consts.tile([128, SJ, h], bf16)
            for j in range(SJ):
                nc.scalar.dma_start(
                    out=waT[:, j, :], in_=waT_in[j * 128 : (j + 1) * 128, :]
                )

            # qT strips [j=128, bpc] via PE transpose of q_bf
            qT = consts.tile([128, SJ, bpc], bf16)
            for j in range(SJ):
                ptr = ps_setup.tile([128, bpc], bf16, tag="setup")
                nc.tensor.transpose(
                    out=ptr,
                    in_=q_bf[:, j * 128 : (j + 1) * 128],
                    identity=ident[:bpc, :bpc],
                )
                nc.vector.tensor_copy(out=qT[:, j, :], in_=ptr)

            # Va columns [h=128, SM] via PE transpose of the bf16 row
            va_cols = consts.tile([128, SM], bf16)
            for m in range(SM):
                vtr = ps_setup.tile([128, 1], bf16, tag="setup")
                nc.tensor.transpose(
                    out=vtr,
                    in_=va_bf[:1, m * 128 : (m + 1) * 128],
                    identity=ident[:1, :1],
                )
                nc.vector.tensor_copy(out=va_cols[:, m : m + 1], in_=vtr)

            # ---------------- keys pipeline helpers ----------------
            # strips for one chunk: cast-loads f32->bf16 into SBUF (reused by
            # the context matmul), then stores to the DRAM scratch in natural
            # layout. All on gpsimd (only SWDGE can cast); loads are batched
            # before stores so the transfers overlap despite the in-order
            # queue (store i only head-of-line-blocks after load i landed).
            def load_strips(knat, b, c, store=True):
                strips = []
                for i in range(SPC):
                    si = c * SPC + i
                    ks = kcache.tile([128, d2], bf16, tag="ks", name=f"ks_{b}_{si}")
                    nc.gpsimd.dma_start(
                        out=ks, in_=keys_in[si * 128 : (si + 1) * 128, b, :]
                    )
                    strips.append(ks)
                if store:
                    # only the bf16-path columns (d >= nf8) round-trip through
                    # DRAM; the fp8 strips are PE-transposed from SBUF
                    for i in range(SPC):
                        si = c * SPC + i
                        nc.gpsimd.dma_start(
                            out=knat[si * 128 : (si + 1) * 128, :],
                            in_=strips[i][:, nf8 * 128 :],
                        )
                return strips

            def load_kts(knat, b, g):
                # one transposed tile per d covering LG chunks of s: the
                # ~1.3us fixed cost per xbar instruction serializes on the
                # Sync queue, so fewer/bigger transposes keep PE fed
                # bf16 strips only (d >= nf8, index shifted by nf8).
                # All on the sync queue: DMA on the scalar queue races
                # with the activation stream (observed corruption whenever
                # bulk DMA shares the Activation engine's queue mid-kernel)
                kts = []
                for d in range(nf8, SD):
                    kt = ktp.tile(
                        [128, LG * schunk], bf16, tag="kt", name=f"kt_{b}_{g}_{d}"
                    )
                    nc.sync.dma_start(
                        out=kt,
                        in_=knat[
                            g * LG * schunk : (g + 1) * LG * schunk,
                            (d - nf8) * 128 : (d - nf8 + 1) * 128,
                        ],
                        transpose=True,
                    )
                    kts.append(kt)
                return kts

            def pe_f8(chunk_strips, tagname):
                # fp8 kT pair-tiles [128, 2, s] for the DoubleRow 2x matmul
                # path, built by PE transposes straight from the SBUF strips
                # (no DRAM round trip, no wait on the xbar queue) and packed
                # by a single casting vector copy per k-tile
                f8ts = []
                for pr in range(NPAIR):
                    f8t = f8p.tile(
                        [128, 2, LG * schunk], fp8, tag="f8",
                        name=f"f8_{tagname}_{pr}",
                    )
                    for t in range(2):
                        d = 2 * pr + t
                        ptr = ps_uk.tile([128, 2 * SPC * 128], bf16, tag="puk")
                        for cc in range(len(chunk_strips)):
                            for i in range(SPC):
                                nc.tensor.transpose(
                                    out=ptr[
                                        :,
                                        (cc * SPC + i) * 128 : (cc * SPC + i + 1)
                                        * 128,
                                    ],
                                    in_=chunk_strips[cc][i][
                                        :, d * 128 : (d + 1) * 128
                                    ],
                                    identity=ident,
                                )
                        nc.vector.tensor_copy(
                            out=f8t[:, t, :], in_=ptr[:, : LG * schunk]
                        )
                    f8ts.append(f8t)
                return f8ts

            # ---------------- main loop over batches ----------------
            # strips are prefetched PF slots ahead; each kT load-group is
            # issued one slot before its first use so the 16 xbar ops run
            # during the previous chunk's compute.
            # first kT group is built by PE transposes from the SBUF strips
            # (no DRAM round trip): strips land ~12us in, so the first uk
            # matmul can start ~35us instead of waiting ~120us for the xbar
            # chain to clear the startup DMA burst.
            def pe_kts(chunk_strips):
                # chunk_strips: list of LG lists of SPC strips; bf16 path
                # tiles only (the fp8 strips get their own pe_f8 tiles)
                kts = []
                for d in range(nf8, SD):
                    kt = ktp.tile(
                        [128, LG * schunk], bf16, tag="kt", name=f"kt_pe_{d}"
                    )
                    # share the uk PSUM ring: same tag and byte footprint
                    # ([128, 2*SPC*128] bf16 == [128, SPC*128] fp32), so the
                    # transposes triple-buffer without extra banks
                    ptr = ps_uk.tile([128, 2 * SPC * 128], bf16, tag="puk")
                    for cc in range(LG):
                        for i in range(SPC):
                            nc.tensor.transpose(
                                out=ptr[
                                    :,
                                    (cc * SPC + i) * 128 : (cc * SPC + i + 1) * 128,
                                ],
                                in_=chunk_strips[cc][i][
                                    :, d * 128 : (d + 1) * 128
                                ],
                                identity=ident,
                            )
                    nc.vector.tensor_copy(
                        out=kt, in_=ptr[:, : LG * schunk]
                    )
                    kts.append(kt)
                return kts

            seq = [(b, c) for b in range(bpc) for c in range(NCH)]
            PF = 3 if NCH > 2 else 1
            knats = {}
            pending = {}
            pending_kts = {}

            pending_f8 = {}

            knats[0] = dram_kn.tile(
                [s, d2 - nf8 * 128], bf16, tag="knat", name="knat_b0"
            )
            pending[(0, 0)] = load_strips(knats[0], 0, 0, store=False)
            if LG > 1:
                pending[(0, 1)] = load_strips(knats[0], 0, 1, store=False)
                pending_kts[(0, 0)] = pe_kts([pending[(0, 0)], pending[(0, 1)]])
            else:
                pending_kts[(0, 0)] = pe_kts([pending[(0, 0)]])
                if NCH > 1:
                    pending[(0, 1)] = load_strips(knats[0], 0, 1)
            if NPAIR:
                prime_strips = [pending[(0, 0)]]
                if LG > 1:
                    prime_strips.append(pending[(0, 1)])
                pending_f8[(0, 0)] = pe_f8(prime_strips, "g00")
            if NCH > 2:
                pending[(0, 2)] = load_strips(knats[0], 0, 2)

            # bias_cols[:, m, b] = (Wa q_b)[128m:128m+128] + Wa_b + Ua_b (fp32)
            # Emitted AFTER the pe_kts transposes: waT loads pace in behind
            # the strip burst, and these matmuls must not stall the in-order
            # PE queue ahead of the first-group transposes. They complete
            # well before the first tanh needs the bias.
            bias_cols = consts.tile([128, SM, bpc], fp32)
            for m in range(SM):
                pw = ps_setup.tile([128, bpc], fp32, tag="setup")
                for j in range(SJ):
                    nc.tensor.matmul(
                        out=pw,
                        lhsT=waT[:, j, m * 128 : (m + 1) * 128],
                        rhs=qT[:, j, :],
                        start=(j == 0),
                        stop=False,
                    )
                nc.tensor.matmul(
                    out=pw,
                    lhsT=comb_bf[:1, m * 128 : (m + 1) * 128],
                    rhs=ones_bf,
                    start=False,
                    stop=True,
                )
                nc.vector.tensor_copy(out=bias_cols[:, m, :], in_=pw)

            kts_group = None
            for b in range(bpc):
                exp_row = rows.tile([1, s], fp32, tag="exp_row")
                tparts = rows.tile([1, NCH], fp32, tag="tparts")
                ecols = rows.tile([128, NST], bf16, tag="ecols")
                ctx_acc = acc1.tile([1, d2], fp32, tag="ctx_acc")
                for c in range(NCH):
                    f8_todo = None
                    strips = pending.pop((b, c))
                    if c % LG == 0:
                        kts_group = pending_kts.pop((b, c // LG))
                        f8_group = (
                            pending_f8.pop((b, c // LG)) if NPAIR else None
                        )
                    sub = c % LG
                    pos = b * NCH + c
                    # prefetch strips PF slots ahead
                    if pos + PF < len(seq):
                        nb, nxc = seq[pos + PF]
                        if nb not in knats:
                            knats[nb] = dram_kn.tile(
                                [s, d2 - nf8 * 128], bf16, tag="knat",
                                name=f"knat_b{nb}",
                            )
                        pending[(nb, nxc)] = load_strips(knats[nb], nb, nxc)
                    # issue the NEXT slot's kT group (if one starts there)
                    if pos + 1 < len(seq):
                        nb, nxc = seq[pos + 1]
                        if nxc % LG == 0:
                            pending_kts[(nb, nxc // LG)] = load_kts(
                                knats[nb], nb, nxc // LG
                            )
                            if NPAIR:
                                # fp8 tiles come from this group's SBUF
                                # strips; built at the END of this slot so
                                # the in-order PE queue is not parked on
                                # the strip loads issued just above
                                nsl = [pending[(nb, nxc)]]
                                if LG > 1:
                                    nsl.append(pending[(nb, nxc + 1)])
                                f8_todo = (
                                    (nb, nxc // LG), nsl,
                                    f"g{nb}_{nxc // LG}",
                                )
                    # ukT tiles + tanh; score matmuls are deferred until all
                    # tanh tiles exist so the in-order PE queue never waits
                    # on the Scalar engine mid-chunk
                    psc = ps_sc.tile([1, schunk], fp32, tag="psc")
                    ts_list = []
                    for m in range(SM):
                        puk = ps_uk.tile([128, schunk], fp32, tag="puk")
                        for pr in range(NPAIR):
                            nc.tensor.matmul(
                                out=puk,
                                lhsT=ua8[:, pr, :, m * 128 : (m + 1) * 128],
                                rhs=f8_group[pr][
                                    :, :, sub * schunk : (sub + 1) * schunk
                                ],
                                start=(pr == 0),
                                stop=False,
                                perf_mode=mybir.MatmulPerfMode.DoubleRow,
                            )
                        for d in range(nf8, SD):
                            nc.tensor.matmul(
                                out=puk,
                                lhsT=uaT[:, d - nf8, m * 128 : (m + 1) * 128],
                                rhs=kts_group[d - nf8][
                                    :, sub * schunk : (sub + 1) * schunk
                                ],
                                start=(d == nf8 and not NPAIR),
                                stop=(d == SD - 1),
                            )
                        t_sb = tp.tile([128, schunk], bf16, tag="t")
                        nc.scalar.activation(
                            out=t_sb,
                            in_=puk,
                            func=AF.Tanh,
                            bias=bias_cols[:, m, b : b + 1],
                            scale=1.0 / UA_SCALE,
                        )
                        ts_list.append(t_sb)
                    for m in range(SM):
                        nc.tensor.matmul(
                            out=psc,
                            lhsT=va_cols[:, m : m + 1],
                            rhs=ts_list[m],
                            start=(m == 0),
                            stop=(m == SM - 1),
                        )
                    # exp row chunk (no max subtraction; scores are O(1)) and
                    # the chunk's softmax partial sum
                    nc.scalar.activation(
                        out=exp_row[:, c * schunk : (c + 1) * schunk],
                        in_=psc,
                        func=AF.Exp,
                        accum_out=tparts[:, c : c + 1],
                    )
                    # transpose this chunk's scores into columns on PE (tiny)
                    # and exp them -> unnormalized weight columns for context
                    scsb = rows.tile([1, schunk], fp32, tag="scsb")
                    nc.vector.tensor_copy(out=scsb, in_=psc)
                    pscT = ps_setup.tile([128, SPC], fp32, tag="setup")
                    for g in range(SPC):
                        nc.tensor.transpose(
                            out=pscT[:, g : g + 1],
                            in_=scsb[:1, g * 128 : (g + 1) * 128],
                            identity=ident_f32[:1, :1],
                        )
                    nc.scalar.activation(
                        out=ecols[:, c * SPC : (c + 1) * SPC],
                        in_=pscT,
                        func=AF.Exp,
                    )
                    # context partial for this chunk's strips (normalized at
                    # the end of the batch): ctx += sum_si e[si] * k[si, :]
                    for jd in range(NDC):
                        pcx = ps_cx.tile([1, min(512, d2)], fp32, tag="pcx")
                        for i in range(SPC):
                            nc.tensor.matmul(
                                out=pcx,
                                lhsT=ecols[:, c * SPC + i : c * SPC + i + 1],
                                rhs=strips[i][:, jd * 512 : (jd + 1) * 512],
                                start=(i == 0),
                                stop=(i == SPC - 1),
                            )
                        if c == 0:
                            nc.vector.tensor_copy(
                                out=ctx_acc[:, jd * 512 : (jd + 1) * 512], in_=pcx
                            )
                        else:
                            nc.vector.tensor_add(
                                out=ctx_acc[:, jd * 512 : (jd + 1) * 512],
                                in0=ctx_acc[:, jd * 512 : (jd + 1) * 512],
                                in1=pcx,
                            )
                    if f8_todo is not None:
                        fkey, fsl, ftag = f8_todo
                        pending_f8[fkey] = pe_f8(fsl, ftag)
                # softmax denominator; normalize weights + context, write out
                tsum = rows.tile([1, 1], fp32, tag="tsum")
                nc.vector.reduce_sum(
                    out=tsum, in_=tparts, axis=mybir.AxisListType.X
                )
                invt = rows.tile([1, 1], fp32, tag="invt")
                nc.vector.reciprocal(out=invt, in_=tsum)
                nc.vector.tensor_scalar_mul(out=exp_row, in0=exp_row, scalar1=invt)
                nc.scalar.dma_start(out=w_out[b : b + 1, :], in_=exp_row)
                nc.vector.tensor_scalar_mul(out=ctx_acc, in0=ctx_acc, scalar1=invt)
                nc.scalar.dma_start(out=ctx_out[b : b + 1, :], in_=ctx_acc)

    nc.compile()
    return nc


def _get_nc():
    if "nc" not in _CACHE:
        _CACHE["nc"] = _build()
    return _CACHE["nc"]


def _prep_weights(Wa_w, Wa_b, Ua_w, Ua_b, Va_w, nf8=NF8):
    """Host-side weight pre-layout: transpose + cast once. uaT is pre-scaled
    by UA_SCALE (exponent-only in bf16, exact) so the fp8 copy of its leading
    strips quantizes in e4m3's normal range; the kernel's tanh un-scales."""
    import ml_dtypes

    bf = ml_dtypes.bfloat16
    f8 = ml_dtypes.float8_e4m3
    h = Wa_w.shape[0]
    uaT_s = np.ascontiguousarray(
        np.asarray(Ua_w, dtype=np.float32).T * UA_SCALE
    )
    return {
        "uaT": uaT_s.astype(bf),
        "uaT8": uaT_s[: nf8 * 128].astype(f8),
        "waT": np.ascontiguousarray(np.asarray(Wa_w, dtype=np.float32).T).astype(bf),
        "va": np.asarray(Va_w, dtype=np.float32).reshape(1, h).astype(bf),
        "comb": (
            np.asarray(Wa_b, dtype=np.float32) + np.asarray(Ua_b, dtype=np.float32)
        ).reshape(1, h).astype(bf),
    }


def _make_in_maps(inputs):
    q_last = np.ascontiguousarray(
        np.asarray(inputs["query"], dtype=np.float32)[:, -1, :]
    )  # [B, H]
    keys = np.asarray(inputs["keys"], dtype=np.float32)  # [S, B, 2H]
    weights = _prep_weights(
        inputs["Wa_w"], inputs["Wa_b"], inputs["Ua_w"], inputs["Ua_b"], inputs["Va_w"]
    )
    in_maps = []
    for c in range(NCORES):
        b0 = c * BPC
        in_maps.append(
            {
                "q": np.ascontiguousarray(q_last[b0 : b0 + BPC]),
                "keys": np.ascontiguousarray(keys[:, b0 : b0 + BPC, :]),
                **weights,
            }
        )
    return in_maps


def run(inputs, trace=False, **kwargs):
    """Run on all 8 cores; returns ((context, weights), BassKernelResults)."""
    from concourse.bass_utils import run_bass_kernel_spmd

    nc = _get_nc()
    in_maps = _make_in_maps(inputs)
    res = run_bass_kernel_spmd(
        nc, in_maps, core_ids=list(range(NCORES)), trace=trace, **kwargs
    )
    context = np.empty((B, 1, D2), dtype=np.float32)
    weights = np.empty((B, 1, S), dtype=np.float32)
    for c in range(NCORES):
        b0 = c * BPC
        context[b0 : b0 + BPC, 0, :] = res.results[c]["ctx"]
        weights[b0 : b0 + BPC, 0, :] = res.results[c]["wts"]
    return (context, weights), res


def kernel(**inputs):
    out, _ = run(inputs)
    return out


# revision 29
# speedup vs baseline: 1.2179x; 1.0083x over previous
"""Bahdanau additive attention kernel for Trainium2 (8 NeuronCores, SPMD).

Problem (hardcoded): B=32, Tq=4, S=2048, H=1024, 2H=2048, fp32 inputs.
  q  = query[:, -1, :]                      [B, H]
  k  = transpose(keys, (1, 0, 2))           [B, S, 2H]
  wq = q @ Wa_w.T + Wa_b                    [B, H]
  uk = k @ Ua_w.T + Ua_b                    [B, S, H]
  sc = tanh(wq[:, None, :] + uk) @ Va_w.T   [B, S]   (+ Va_b, which softmax cancels)
  w  = softmax(sc, axis=-1)                 [B, S]
  ctx = w @ k                               [B, 2H]
  returns (ctx [B,1,2H], w [B,1,S])

Sharding: data-parallel over batch. 8 cores x 4 batches each; weights
replicated; no cross-core communication.

Host prep: weights are pre-transposed and cast to bf16 on the host
(uaT = Ua_w.T, waT = Wa_w.T, va, comb = Wa_b + Ua_b). This removes the
on-device SWDGE cast + xbar transpose chain for weights that serialized
~120us of startup in front of the first uk matmul.

Per-core dataflow (all matmuls bf16 with fp32 PSUM accumulation):
  - uaT/waT strips are direct-loaded [128, strip, h] on the scalar queue.
  - keys strips [128, 2H] are cast-loaded f32->bf16 on gpsimd (kept in
    SBUF for the context matmul), stored to a DRAM scratch (also gpsimd,
    loads batched before stores so transfers overlap), and read back
    transposed ([d=128, s=LG*schunk] tiles) via the DMA xbar on the sync
    queue. kT load-groups are issued one slot ahead of use with a
    two-group buffer pool so the sync queue stays a chunk ahead of PE.
  - ukT tiles [h=128, s=512] accumulate in PSUM; ScalarE applies
    tanh(. + bias[h]) where bias = wq[b] + Wa_b + Ua_b folded per-partition.
  - scores via PE with Va columns as the 1-wide stationary operand.
  - softmax without max-subtraction (scores are O(1)); exp on ScalarE with
    free-dim accumulate for the denominator.
  - context via PE with normalized-late weights columns (PE-transposed
    score chunks) against the cached bf16 keys strips.
"""

import numpy as np

B, TQ, S, H = 32, 4, 2048, 1024
D2 = 2 * H
NCORES = 8
BPC = B // NCORES  # batches per core
NF8 = 6       # leading d-strips done in fp8 e4m3 (DoubleRow, 2x PE rate)
UA_SCALE = 64.0  # Ua pre-scale so fp8 quantization stays in normal range

_CACHE = {}


def _build(s=S, h=H, bpc=BPC, schunk=512, nf8=NF8):
    """Build the per-core Bass module. Parameterized so a scaled-down config
    can run in CoreSim; the shipped kernel uses the defaults."""
    from contextlib import ExitStack

    import concourse.bacc as bacc
    import concourse.mybir as mybir
    import concourse.tile as tile
    from concourse.masks import make_identity

    fp32 = mybir.dt.float32
    bf16 = mybir.dt.bfloat16
    fp8 = mybir.dt.float8e4
    AF = mybir.ActivationFunctionType
    d2 = 2 * h
    SD = d2 // 128        # contraction strips for uk (d on partitions)
    SM = h // 128         # h tiles (uk output partitions / Va strips)
    SJ = h // 128         # contraction strips for wq
    NCH = s // schunk     # score chunks per batch
    SPC = schunk // 128   # keys strips per chunk
    NDC = max(1, d2 // 512)  # context output chunks
    NST = s // 128        # keys strips per batch
    LG = 2 if NCH % 2 == 0 else 1  # chunks per kT load-group
    NPAIR = nf8 // 2      # fp8 DoubleRow d-strip pairs (2x PE rate)
    assert nf8 % 2 == 0 and nf8 <= SD

    nc = bacc.Bacc(
        "TRN2", target_bir_lowering=False, enable_partition_id=False
    )

    q_in = nc.dram_tensor("q", [bpc, h], fp32, kind="ExternalInput").ap()
    keys_in = nc.dram_tensor("keys", [s, bpc, d2], fp32, kind="ExternalInput").ap()
    uaT_in = nc.dram_tensor("uaT", [d2, h], bf16, kind="ExternalInput").ap()
    waT_in = nc.dram_tensor("waT", [h, h], bf16, kind="ExternalInput").ap()
    ua8_in = nc.dram_tensor("uaT8", [nf8 * 128, h], fp8, kind="ExternalInput").ap()
    va_in = nc.dram_tensor("va", [1, h], bf16, kind="ExternalInput").ap()
    comb_in = nc.dram_tensor("comb", [1, h], bf16, kind="ExternalInput").ap()
    ctx_out = nc.dram_tensor("ctx", [bpc, d2], fp32, kind="ExternalOutput").ap()
    w_out = nc.dram_tensor("wts", [bpc, s], fp32, kind="ExternalOutput").ap()

    with tile.TileContext(nc) as tc:
        with ExitStack() as ctx:
            consts = ctx.enter_context(tc.tile_pool(name="consts", bufs=1))
            dram_kn = ctx.enter_context(
                tc.tile_pool(name="dram_kn", bufs=bpc, space="DRAM")
            )
            kcache = ctx.enter_context(
                tc.tile_pool(name="kcache", bufs=4 * SPC)
            )
            ktp = ctx.enter_context(
                tc.tile_pool(name="ktp", bufs=2 * (SD - nf8))
            )
            f8p = ctx.enter_context(
                tc.tile_pool(name="f8p", bufs=2 * max(NPAIR, 1))
            )
            tp = ctx.enter_context(tc.tile_pool(name="tp", bufs=SM + 1))
            rows = ctx.enter_context(tc.tile_pool(name="rows", bufs=1))
            acc1 = ctx.enter_context(tc.tile_pool(name="acc1", bufs=1))
            ps_setup = ctx.enter_context(
                tc.tile_pool(name="ps_setup", bufs=1, space="PSUM")
            )
            ps_uk = ctx.enter_context(tc.tile_pool(name="ps_uk", bufs=3, space="PSUM"))
            ps_sc = ctx.enter_context(tc.tile_pool(name="ps_sc", bufs=2, space="PSUM"))
            ps_cx = ctx.enter_context(tc.tile_pool(name="ps_cx", bufs=2, space="PSUM"))

            # ---------------- one-time setup ----------------
            ident = consts.tile([128, 128], bf16)
            make_identity(nc, ident)
            # only the [1,1] corner is ever used (pscT row transposes)
            ident_f32 = consts.tile([1, 1], fp32)
            nc.vector.memset(ident_f32, 1.0)

            # q cast-load first on gpsimd (gates qT -> bias_cols).
            q_bf = consts.tile([bpc, h], bf16)
            nc.gpsimd.dma_start(out=q_bf, in_=q_in)

            # Small bf16 vectors + transposed weights direct loads (scalar
            # queue; sync is reserved for the keys xbar transposes).
            va_bf = consts.tile([1, h], bf16)
            nc.scalar.dma_start(out=va_bf, in_=va_in)
            comb_bf = consts.tile([1, h], bf16)
            nc.scalar.dma_start(out=comb_bf, in_=comb_in)
            ones_bf = consts.tile([1, bpc], bf16)
            nc.vector.memset(ones_bf, 1.0)

            # uaT[:, d, :] = Ua[:, 128d:128d+128].T  -> [dpart=128, h]
            # uaT rides the sync queue (idle until the first xbar group at
            # ~60us and directly gates the first uk matmuls); waT on scalar.
            # only the strips the bf16 path still needs (d >= nf8)
            uaT = consts.tile([128, SD - nf8, h], bf16)
            for d in range(nf8, SD):
                nc.sync.dma_start(
                    out=uaT[:, d - nf8, :],
                    in_=uaT_in[d * 128 : (d + 1) * 128, :],
                )
            ua8 = consts.tile([128, NPAIR, 2, h], fp8)
            for pr in range(NPAIR):
                for t in range(2):
                    nc.sync.dma_start(
                        out=ua8[:, pr, t, :],
                        in_=ua8_in[(2 * pr + t) * 128 : (2 * pr + t + 1) * 128, :],
                    )
            waT = consts.tile([128, SJ, h], bf16)
            for j in range(SJ):
                nc.scalar.dma_start(
                    out=waT[:, j, :], in_=waT_in[j * 128 : (j + 1) * 128, :]
                )

            # qT strips [j=128, bpc] via PE transpose of q_bf
            qT = consts.tile([128, SJ, bpc], bf16)
            for j in range(SJ):
                ptr = ps_setup.tile([128, bpc], bf16, tag="setup")
                nc.tensor.transpose(
                    out=ptr,
                    in_=q_bf[:, j * 128 : (j + 1) * 128],
                    identity=ident[:bpc, :bpc],
                )
                nc.vector.tensor_copy(out=qT[:, j, :], in_=ptr)

            # Va columns [h=128, SM] via PE transpose of the bf16 row
            va_cols = consts.tile([128, SM], bf16)
            for m in range(SM):
                vtr = ps_setup.tile([128, 1], bf16, tag="setup")
                nc.tensor.transpose(
                    out=vtr,
                    in_=va_bf[:1, m * 128 : (m + 1) * 128],
                    identity=ident[:1, :1],
                )
                nc.vector.tensor_copy(out=va_cols[:, m : m + 1], in_=vtr)

            # ---------------- keys pipeline helpers ----------------
            # strips for one chunk: cast-loads f32->bf16 into SBUF (reused by
            # the context matmul), then stores to the DRAM scratch in natural
            # layout. All on gpsimd (only SWDGE can cast); loads are batched
            # before stores so the transfers overlap despite the in-order
            # queue (store i only head-of-line-blocks after load i landed).
            def load_strips(knat, b, c, store=True):
                strips = []
                for i in range(SPC):
                    si = c * SPC + i
                    ks = kcache.tile([128, d2], bf16, tag="ks", name=f"ks_{b}_{si}")
                    nc.gpsimd.dma_start(
                        out=ks, in_=keys_in[si * 128 : (si + 1) * 128, b, :]
                    )
                    strips.append(ks)
                if store:
                    # only the bf16-path columns (d >= nf8) round-trip through
                    # DRAM; the fp8 strips are PE-transposed from SBUF
                    for i in range(SPC):
                        si = c * SPC + i
                        nc.gpsimd.dma_start(
                            out=knat[si * 128 : (si + 1) * 128, :],
                            in_=strips[i][:, nf8 * 128 :],
                        )
                return strips

            def load_kts(knat, b, g):
                # one transposed tile per d covering LG chunks of s: the
                # ~1.3us fixed cost per xbar instruction serializes on the
                # Sync queue, so fewer/bigger transposes keep PE fed
                # bf16 strips only (d >= nf8, index shifted by nf8).
                # All on the sync queue: DMA on the scalar queue races
                # with the activation stream (observed corruption whenever
                # bulk DMA shares the Activation engine's queue mid-kernel)
                kts = []
                for d in range(nf8, SD):
                    kt = ktp.tile(
                        [128, LG * schunk], bf16, tag="kt", name=f"kt_{b}_{g}_{d}"
                    )
                    nc.sync.dma_start(
                        out=kt,
                        in_=knat[
                            g * LG * schunk : (g + 1) * LG * schunk,
                            (d - nf8) * 128 : (d - nf8 + 1) * 128,
                        ],
                        transpose=True,
                    )
                    kts.append(kt)
                return kts

            def pe_f8(chunk_strips, tagname):
                # fp8 kT pair-tiles [128, 2, s] for the DoubleRow 2x matmul
                # path, built by PE transposes straight from the SBUF strips
                # (no DRAM round trip, no wait on the xbar queue) and packed
                # by a single casting vector copy per k-tile
                f8ts = []
                for pr in range(NPAIR):
                    f8t = f8p.tile(
                        [128, 2, LG * schunk], fp8, tag="f8",
                        name=f"f8_{tagname}_{pr}",
                    )
                    for t in range(2):
                        d = 2 * pr + t
                        ptr = ps_uk.tile([128, 2 * SPC * 128], bf16, tag="puk")
                        for cc in range(len(chunk_strips)):
                            for i in range(SPC):
                                nc.tensor.transpose(
                                    out=ptr[
                                        :,
                                        (cc * SPC + i) * 128 : (cc * SPC + i + 1)
                                        * 128,
                                    ],
                                    in_=chunk_strips[cc][i][
                                        :, d * 128 : (d + 1) * 128
                                    ],
                                    identity=ident,
                                )
                        nc.vector.tensor_copy(
                            out=f8t[:, t, :], in_=ptr[:, : LG * schunk]
                        )
                    f8ts.append(f8t)
                return f8ts

            # ---------------- main loop over batches ----------------
            # strips are prefetched PF slots ahead; each kT load-group is
            # issued one slot before its first use so the 16 xbar ops run
            # during the previous chunk's compute.
            # first kT group is built by PE transposes from the SBUF strips
            # (no DRAM round trip): strips land ~12us in, so the first uk
            # matmul can start ~35us instead of waiting ~120us for the xbar
            # chain to clear the startup DMA burst.
            def pe_kts(chunk_strips):
                # chunk_strips: list of LG lists of SPC strips; bf16 path
                # tiles only (the fp8 strips get their own pe_f8 tiles)
                kts = []
                for d in range(nf8, SD):
                    kt = ktp.tile(
                        [128, LG * schunk], bf16, tag="kt", name=f"kt_pe_{d}"
                    )
                    # share the uk PSUM ring: same tag and byte footprint
                    # ([128, 2*SPC*128] bf16 == [128, SPC*128] fp32), so the
                    # transposes triple-buffer without extra banks
                    ptr = ps_uk.tile([128, 2 * SPC * 128], bf16, tag="puk")
                    for cc in range(LG):
                        for i in range(SPC):
                            nc.tensor.transpose(
                                out=ptr[
                                    :,
                                    (cc * SPC + i) * 128 : (cc * SPC + i + 1) * 128,
                                ],
                                in_=chunk_strips[cc][i][
                                    :, d * 128 : (d + 1) * 128
                                ],
                                identity=ident,
                            )
                    nc.vector.tensor_copy(
                        out=kt, in_=ptr[:, : LG * schunk]
                    )
                    kts.append(kt)
                return kts

            seq = [(b, c) for b in range(bpc) for c in range(NCH)]
            PF = 3 if NCH > 2 else 1
            knats = {}
            pending = {}
            pending_kts = {}

            pending_f8 = {}

            knats[0] = dram_kn.tile(
                [s, d2 - nf8 * 128], bf16, tag="knat", name="knat_b0"
            )
            pending[(0, 0)] = load_strips(knats[0], 0, 0, store=False)
            if LG > 1:
                pending[(0, 1)] = load_strips(knats[0], 0, 1, store=False)
                pending_kts[(0, 0)] = pe_kts([pending[(0, 0)], pending[(0, 1)]])
            else:
                pending_kts[(0, 0)] = pe_kts([pending[(0, 0)]])
                if NCH > 1:
                    pending[(0, 1)] = load_strips(knats[0], 0, 1)
            if NPAIR:
                prime_strips = [pending[(0, 0)]]
                if LG > 1:
                    prime_strips.append(pending[(0, 1)])
                pending_f8[(0, 0)] = pe_f8(prime_strips, "g00")
            if NCH > 2:
                pending[(0, 2)] = load_strips(knats[0], 0, 2)

            # bias_cols[:, m, b] = (Wa q_b)[128m:128m+128] + Wa_b + Ua_b (fp32)
            # Emitted AFTER the pe_kts transposes: waT loads pace in behind
            # the strip burst, and these matmuls must not stall the in-order
            # PE queue ahead of the first-group transposes. They complete
            # well before the first tanh needs the bias.
            bias_cols = consts.tile([128, SM, bpc], fp32)
            for m in range(SM):
                pw = ps_setup.tile([128, bpc], fp32, tag="setup")
                for j in range(SJ):
                    nc.tensor.matmul(
                        out=pw,
                        lhsT=waT[:, j, m * 128 : (m + 1) * 128],
                        rhs=qT[:, j, :],
                        start=(j == 0),
                        stop=False,
                    )
                nc.tensor.matmul(
                    out=pw,
                    lhsT=comb_bf[:1, m * 128 : (m + 1) * 128],
                    rhs=ones_bf,
                    start=False,
                    stop=True,
                )
                nc.vector.tensor_copy(out=bias_cols[:, m, :], in_=pw)

            kts_group = None
            for b in range(bpc):
                exp_row = rows.tile([1, s], fp32, tag="exp_row")
                tparts = rows.tile([1, NCH], fp32, tag="tparts")
                ecols = rows.tile([128, NST], bf16, tag="ecols")
                ctx_acc = acc1.tile([1, d2], fp32, tag="ctx_acc")
                for c in range(NCH):
                    f8_todo = None
                    strips = pending.pop((b, c))
                    if c % LG == 0:
                        kts_group = pending_kts.pop((b, c // LG))
                        f8_group = (
                            pending_f8.pop((b, c // LG)) if NPAIR else None
                        )
                    sub = c % LG
                    pos = b * NCH + c
                    # prefetch strips PF slots ahead
                    if pos + PF < len(seq):
                        nb, nxc = seq[pos + PF]
                        if nb not in knats:
                            knats[nb] = dram_kn.tile(
                                [s, d2 - nf8 * 128], bf16, tag="knat",
                                name=f"knat_b{nb}",
                            )
                        pending[(nb, nxc)] = load_strips(knats[nb], nb, nxc)
                    # issue the NEXT slot's kT group (if one starts there)
                    if pos + 1 < len(seq):
                        nb, nxc = seq[pos + 1]
                        if nxc % LG == 0:
                            pending_kts[(nb, nxc // LG)] = load_kts(
                                knats[nb], nb, nxc // LG
                            )
                            if NPAIR:
                                # fp8 tiles come from this group's SBUF
                                # strips; built at the END of this slot so
                                # the in-order PE queue is not parked on
                                # the strip loads issued just above
                                nsl = [pending[(nb, nxc)]]
                                if LG > 1:
                                    nsl.append(pending[(nb, nxc + 1)])
                                f8_todo = (
                                    (nb, nxc // LG), nsl,
                                    f"g{nb}_{nxc // LG}",
                                )
                    # ukT tiles + tanh; score matmuls are deferred until all
                    # tanh tiles exist so the in-order PE queue never waits
                    # on the Scalar engine mid-chunk
                    psc = ps_sc.tile([1, schunk], fp32, tag="psc")
                    ts_list = []
                    for m in range(SM):
                        puk = ps_uk.tile([128, schunk], fp32, tag="puk")
                        for pr in range(NPAIR):
                            nc.tensor.matmul(
                                out=puk,
                                lhsT=ua8[:, pr, :, m * 128 : (m + 1) * 128],
                                rhs=f8_group[pr][
                                    :, :, sub * schunk : (sub + 1) * schunk
                                ],
                                start=(pr == 0),
                                stop=False,
                                perf_mode=mybir.MatmulPerfMode.DoubleRow,
                            )
                        for d in range(nf8, SD):
                            nc.tensor.matmul(
                                out=puk,
                                lhsT=uaT[:, d - nf8, m * 128 : (m + 1) * 128],
                                rhs=kts_group[d - nf8][
                                    :, sub * schunk : (sub + 1) * schunk
                                ],
                                start=(d == nf8 and not NPAIR),
                                stop=(d == SD - 1),
                            )
                        t_sb = tp.tile([128, schunk], bf16, tag="t")
                        nc.scalar.activation(
                            out=t_sb,
                            in_=puk,
                            func=AF.Tanh,
                            bias=bias_cols[:, m, b : b + 1],
                            scale=1.0 / UA_SCALE,
                        )
                        ts_list.append(t_sb)
                    for m in range(SM):
                        nc.tensor.matmul(
                            out=psc,
                            lhsT=va_cols[:, m : m + 1],
                            rhs=ts_list[m],
                            start=(m == 0),
                            stop=(m == SM - 1),
                        )
                    # exp row chunk (no max subtraction; scores are O(1)) and
                    # the chunk's softmax partial sum
                    nc.scalar.activation(
                        out=exp_row[:, c * schunk : (c + 1) * schunk],
                        in_=psc,
                        func=AF.Exp,
                        accum_out=tparts[:, c : c + 1],
                    )
                    # transpose this chunk's scores into columns on PE (tiny)
                    # and exp them -> unnormalized weight columns for context
                    scsb = rows.tile([1, schunk], fp32, tag="scsb")
                    nc.vector.tensor_copy(out=scsb, in_=psc)
                    pscT = ps_setup.tile([128, SPC], fp32, tag="setup")
                    for g in range(SPC):
                        nc.tensor.transpose(
                            out=pscT[:, g : g + 1],
                            in_=scsb[:1, g * 128 : (g + 1) * 128],
                            identity=ident_f32[:1, :1],
                        )
                    nc.scalar.activation(
                        out=ecols[:, c * SPC : (c + 1) * SPC],
                        in_=pscT,
                        func=AF.Exp,
                    )
                    # context partial for this chunk's strips (normalized at
                    # the end of the batch): ctx += sum_si e[si] * k[si, :]
                    for jd in range(NDC):
                        pcx = ps_cx.tile([1, min(512, d2)], fp32, tag="pcx")
                        for i in range(SPC):
                            nc.tensor.matmul(
                                out=pcx,
                                lhsT=ecols[:, c * SPC + i : c * SPC + i + 1],
                                rhs=strips[i][:, jd * 512 : (jd + 1) * 512],
                                start=(i == 0),
                                stop=(i == SPC - 1),
                            )
                        if c == 0:
                            nc.vector.tensor_copy(
                                out=ctx_acc[:, jd * 512 : (jd + 1) * 512], in_=pcx
                            )
                        else:
                            nc.vector.tensor_add(
                                out=ctx_acc[:, jd * 512 : (jd + 1) * 512],
                                in0=ctx_acc[:, jd * 512 : (jd + 1) * 512],
                                in1=pcx,
                            )
                    if f8_todo is not None:
                        fkey, fsl, ftag = f8_todo
                        pending_f8[fkey] = pe_f8(fsl, ftag)
                # softmax denominator; normalize weights + context, write out
                tsum = rows.tile([1, 1], fp32, tag="tsum")
                nc.vector.reduce_sum(
                    out=tsum, in_=tparts, axis=mybir.AxisListType.X
                )
                invt = rows.tile([1, 1], fp32, tag="invt")
                nc.vector.reciprocal(out=invt, in_=tsum)
                nc.vector.tensor_scalar_mul(out=exp_row, in0=exp_row, scalar1=invt)
                nc.scalar.dma_start(out=w_out[b : b + 1, :], in_=exp_row)
                nc.vector.tensor_scalar_mul(out=ctx_acc, in0=ctx_acc, scalar1=invt)
                nc.scalar.dma_start(out=ctx_out[b : b + 1, :], in_=ctx_acc)

    nc.compile()
    return nc


def _get_nc():
    if "nc" not in _CACHE:
        _CACHE["nc"] = _build()
    return _CACHE["nc"]


def _prep_weights(Wa_w, Wa_b, Ua_w, Ua_b, Va_w, nf8=NF8):
    """Host-side weight pre-layout: transpose + cast once. uaT is pre-scaled
    by UA_SCALE (exponent-only in bf16, exact) so the fp8 copy of its leading
    strips quantizes in e4m3's normal range; the kernel's tanh un-scales."""
    import ml_dtypes

    bf = ml_dtypes.bfloat16
    f8 = ml_dtypes.float8_e4m3
    h = Wa_w.shape[0]
    uaT_s = np.ascontiguousarray(
        np.asarray(Ua_w, dtype=np.float32).T * UA_SCALE
    )
    return {
        "uaT": uaT_s.astype(bf),
        "uaT8": uaT_s[: nf8 * 128].astype(f8),
        "waT": np.ascontiguousarray(np.asarray(Wa_w, dtype=np.float32).T).astype(bf),
        "va": np.asarray(Va_w, dtype=np.float32).reshape(1, h).astype(bf),
        "comb": (
            np.asarray(Wa_b, dtype=np.float32) + np.asarray(Ua_b, dtype=np.float32)
        ).reshape(1, h).astype(bf),
    }


def _make_in_maps(inputs):
    q_last = np.ascontiguousarray(
        np.asarray(inputs["query"], dtype=np.float32)[:, -1, :]
    )  # [B, H]
    keys = np.asarray(inputs["keys"], dtype=np.float32)  # [S, B, 2H]
    weights = _prep_weights(
        inputs["Wa_w"], inputs["Wa_b"], inputs["Ua_w"], inputs["Ua_b"], inputs["Va_w"]
    )
    in_maps = []
    for c in range(NCORES):
        b0 = c * BPC
        in_maps.append(
            {
                "q": np.ascontiguousarray(q_last[b0 : b0 + BPC]),
                "keys": np.ascontiguousarray(keys[:, b0 : b0 + BPC, :]),
                **weights,
            }
        )
    return in_maps


def run(inputs, trace=False, **kwargs):
    """Run on all 8 cores; returns ((context, weights), BassKernelResults)."""
    from concourse.bass_utils import run_bass_kernel_spmd

    nc = _get_nc()
    in_maps = _make_in_maps(inputs)
    res = run_bass_kernel_spmd(
        nc, in_maps, core_ids=list(range(NCORES)), trace=trace, **kwargs
    )
    context = np.empty((B, 1, D2), dtype=np.float32)
    weights = np.empty((B, 1, S), dtype=np.float32)
    for c in range(NCORES):
        b0 = c * BPC
        context[b0 : b0 + BPC, 0, :] = res.results[c]["ctx"]
        weights[b0 : b0 + BPC, 0, :] = res.results[c]["wts"]
    return (context, weights), res


def kernel(**inputs):
    out, _ = run(inputs)
    return out
